# revision 1
# baseline (speedup 1.0000x reference)
"""
Causal self-attention (single head) on 8 trn2 NeuronCores.

Problem: x[4, 2048, 1024], Wq/Wk/Wv[1024, 1024] (torch Linear layout [d_out, d_in]).
    q/k/v = x @ W.T ; out = softmax(mask(q k^T) / 32) @ v

Sharding (no collectives, uniform SPMD program):
  core c -> batch b = c // 2, role r = c % 2.
  Both cores of a pair compute K/V projections for the full 2048-row
  sequence of their batch (duplicated work, ~26% extra PE time, avoids
  any cross-core communication).
  Query rows are split between the pair in 4 i-blocks of 256 rows,
  chosen so both roles see the same per-slot causal extents after
  padding to uniform j-tile counts JT_SLOTS = [4, 8, 12, 16]:
     r=0: starts [0, 768, 1024, 1792]  (actual jt 2, 8, 10, 16)
     r=1: starts [256, 512, 1280, 1536](actual jt 4, 6, 12, 14)
  Causality inside the padded slots is enforced with a per-core
  "delta" input: keep score[j, i] iff (jj - ii) <= delta(slot, t);
  delta = I0_global - 128 * t.  Only the last 4 j-tiles of each slot
  need the mask (earlier tiles are all-keep for both roles).

Layouts (all on-chip matmuls contract over the partition dim):
  xT   [d, s]   host-pretransposed  (k/v projections)
  xqT  [d, i_local] host-packed q-rows, pretransposed (q projection)
  WqT/WkT/WvT [d_in, d_out] host-pretransposed
  qT   [o, i_local] DRAM scratch; kT [o, j]: j<1024 SBUF-resident,
       j>=1024 DRAM scratch;  v [j, o] SBUF-resident
  scoresT psum [j 128, i 256] = kT-tile^T @ qT-chunk  (contract o)
  expT = exp(scoresT / 32) * (T0 <= delta)            (T0[jj,ii] = jj-ii)
  ctx  psum [i 128, o 512] += expT-tile^T @ v-tile    (contract j)
  den  psum [i 128, 2]     += expT-tile^T @ ones      (N=2: fp32r needs even N)
  out  = ctx * reciprocal(den)   (natural [i, o] layout, DMA'd out)

All matmuls run as float32r (TF32 mantissa, 1 cycle/row at N>=256 --
4x the plain-fp32 rate); accumulation is fp32 in PSUM.
DMAs are batched via 3D access patterns (one dma_start per 2-4 MB
chunk) because each dma_start costs ~700 ns of sequencer issue time.
"""

import sys

for _p in ("/opt/trn_rl_repo", "/root/.axon_site/_ro/trn_rl_repo"):
    if _p not in sys.path:
        sys.path.append(_p)

import numpy as np

import concourse.bass as bass
import concourse.mybir as mybir
import concourse.tile as tile
from concourse import bacc
from concourse.bass_utils import run_bass_kernel_spmd
import concourse.bass_utils as _bu

# walrus's --enable-ldw-opt=false leaves LDWEIGHTS single-buffered; enable
# the double-buffered weight-load path.
if not getattr(_bu, "_ldw_opt_patched", False):
    _orig_run_command = _bu.run_command

    def _run_command_ldw(cmd, *a, **kw):
        if isinstance(cmd, list):
            cmd = ["--enable-ldw-opt=true" if c == "--enable-ldw-opt=false" else c
                   for c in cmd]
        return _orig_run_command(cmd, *a, **kw)

    _bu.run_command = _run_command_ldw
    _bu._ldw_opt_patched = True

F32 = mybir.dt.float32
F32R = mybir.dt.float32r

B, S, D = 4, 2048, 1024
P = 128
ND = D // P          # 8 d-tiles (projection contraction)
NO = D // P          # 8 o-tiles
IB = 256             # i-block (query block) rows
N_IB = 4
JT_SLOTS = [4, 8, 12, 16]
ROLE_STARTS = {
    0: [0, 768, 1024, 1792],
    1: [256, 512, 1280, 1536],
}
N_CORES = 8


def _mm(nc, out, lhsT, rhs, start, stop):
    nc.tensor.matmul(out, lhsT, rhs, start=start, stop=stop)


def build_program():
    nc = bacc.Bacc(
        "TRN2",
        target_bir_lowering=False,
        debug=False,
        enable_asserts=False,
        num_devices=N_CORES,
    )
    xT = nc.dram_tensor("xT", [D, S], F32R, kind="ExternalInput").ap()
    xqT = nc.dram_tensor("xqT", [D, N_IB * IB], F32R, kind="ExternalInput").ap()
    wqT = nc.dram_tensor("wqT", [D, D], F32R, kind="ExternalInput").ap()
    wkT = nc.dram_tensor("wkT", [D, D], F32R, kind="ExternalInput").ap()
    wvT = nc.dram_tensor("wvT", [D, D], F32R, kind="ExternalInput").ap()
    t0_in = nc.dram_tensor("t0", [P, IB], F32, kind="ExternalInput").ap()
    delta_in = nc.dram_tensor("delta", [P, 16], F32, kind="ExternalInput").ap()
    ones_in = nc.dram_tensor("ones", [P, 2], F32R, kind="ExternalInput").ap()
    out = nc.dram_tensor("out", [N_IB * IB, D], F32, kind="ExternalOutput").ap()

    scale = 1.0 / 32.0  # 1/sqrt(d_v)

    def d_major(ap2d):
        # [ND*P, C] DRAM view -> [P, ND, C] (partition-major 3D AP)
        return ap2d.rearrange("(nd p) c -> p nd c", p=P)

    with tile.TileContext(nc) as tc:
        with (
            tc.tile_pool(name="const", bufs=1) as cpool,
            tc.tile_pool(name="vres", bufs=1) as vpool,
            tc.tile_pool(name="dram", bufs=1, space="DRAM") as dpool,
        ):
            t0_t = cpool.tile([P, IB], F32, tag="t0")
            nc.gpsimd.dma_start(t0_t[:], t0_in[:])
            delta_t = cpool.tile([P, 16], F32, tag="delta")
            nc.gpsimd.dma_start(delta_t[:], delta_in[:])
            ones_t = cpool.tile([P, 2], F32R, tag="ones")
            nc.gpsimd.dma_start(ones_t[:], ones_in[:])

            v_tiles = [
                vpool.tile([P, D], F32R, tag=f"v{j}", name=f"v{j}")
                for j in range(S // P)
            ]
            kT_res = [
                vpool.tile([P, S // 2], F32R, tag=f"kr{o}", name=f"kr{o}")
                for o in range(NO)
            ]
            qT_dram = dpool.tile([D, N_IB * IB], F32R, tag="qTd", name="qTd")
            kT_dram = dpool.tile([D, S // 2], F32R, tag="kTd", name="kTd")

            # ---------------- Phase A: projections ----------------
            with (
                tc.tile_pool(name="xc", bufs=2) as xpool,
                tc.tile_pool(name="psA", bufs=2, space="PSUM") as psA,
            ):
                # PE warm-up while the first loads land, so the HAM clock
                # gate is at 8/8 when real matmuls start.
                warm = xpool.tile([P, 512], F32R, tag="warm", name="warm", bufs=1)
                nc.sync.dma_start(warm[:], xT[0:P, 0:512])
                wps = psA.tile([P, 512], F32, tag="wps", name="wps", bufs=1)
                for w in range(48):
                    _mm(nc, wps[:], warm[:, 0:P], warm[:], start=True, stop=True)

                # One W pool, two tag generations: wq -> even, wv -> odd,
                # wk -> even again (its load overlaps the v stage).
                # Ring placement sequences the HBM traffic by need: wq (sync)
                # + xq (scalar) first; wv/wk sit on the scalar ring BEHIND the
                # first q stores so their transfers overlap q/v compute
                # instead of stealing bandwidth from the critical q loads.
                with tc.tile_pool(name="wp", bufs=1) as wpool:
                    wq_t = wpool.tile([P, ND, D], F32R, tag="wE", name="wq")
                    nc.sync.dma_start(wq_t[:, 0:4, :], d_major(wqT[0:4 * P, :]))
                    nc.gpsimd.dma_start(wq_t[:, 4:8, :], d_major(wqT[4 * P:, :]))
                    wv_t = wpool.tile([P, ND, D], F32R, tag="wO", name="wv")

                    # --- Q projection (spilled to qT_dram) ---
                    with tc.tile_pool(name="stage", bufs=4) as stpool:
                        for sb in range(2):
                            xq = xpool.tile([P, ND, 512], F32R, tag="xc", name=f"xq{sb}")
                            eng = nc.scalar if sb == 0 else nc.gpsimd
                            eng.dma_start(
                                xq[:], d_major(xqT[:, sb * 512:(sb + 1) * 512])
                            )
                            for o in range(NO):
                                pq = psA.tile([P, 512], F32, tag="pk", name=f"pq{sb}_{o}")
                                for d in range(ND):
                                    _mm(nc, pq[:],
                                        wq_t[:, d, o * P:(o + 1) * P], xq[:, d, :],
                                        start=(d == 0), stop=(d == ND - 1))
                                st = stpool.tile([P, 512], F32R, tag="st", name=f"stq{sb}_{o}")
                                nc.vector.tensor_copy(st[:], pq[:])
                                nc.scalar.dma_start(
                                    qT_dram[o * P:(o + 1) * P, sb * 512:(sb + 1) * 512],
                                    st[:],
                                )
                                if sb == 0 and o == 0:
                                    nc.scalar.dma_start(wv_t[:], d_major(wvT))

                    # wk: 2nd generation of the even tag
                    wk_t = wpool.tile([P, ND, D], F32R, tag="wE", name="wk")
                    nc.scalar.dma_start(wk_t[:], d_major(wkT))

                    # qc chunks for phase B: pool opened here so it lands on
                    # the just-freed q-stage staging addresses (readers done
                    # at q end) -- the slot-3/2 prefetches run during kv
                    # instead of stalling at the phase boundary.
                    # --- fused K+V projection over shared x chunks ---
                    # (v + kT-low stay SBUF-resident; kT-high spills to DRAM)
                    with tc.tile_pool(name="stage2", bufs=2) as st2pool:
                        for jb in range(S // 512):
                            xv = xpool.tile([P, ND, 512], F32R, tag="xc", name=f"xv{jb}")
                            nc.gpsimd.dma_start(
                                xv[:], d_major(xT[:, jb * 512:(jb + 1) * 512])
                            )
                            for o in range(NO):
                                pk = psA.tile([P, 512], F32, tag="pk", name=f"pk{jb}_{o}")
                                for d in range(ND):
                                    _mm(nc, pk[:],
                                        wk_t[:, d, o * P:(o + 1) * P], xv[:, d, :],
                                        start=(d == 0), stop=(d == ND - 1))
                                if jb < 2:
                                    nc.vector.tensor_copy(
                                        kT_res[o][:, jb * 512:(jb + 1) * 512], pk[:]
                                    )
                                else:
                                    st = st2pool.tile([P, 512], F32R, tag="st2",
                                                      name=f"stk{jb}_{o}")
                                    nc.vector.tensor_copy(st[:], pk[:])
                                    nc.scalar.dma_start(
                                        kT_dram[o * P:(o + 1) * P,
                                                (jb - 2) * 512:(jb - 1) * 512],
                                        st[:],
                                    )
                            for jj in range(4):
                                jt = jb * 4 + jj
                                for ob in range(2):
                                    pv = psA.tile([P, 512], F32, tag="pv", name=f"pv{jt}_{ob}")
                                    for d in range(ND):
                                        _mm(nc, pv[:],
                                            xv[:, d, jj * P:(jj + 1) * P],
                                            wv_t[:, d, ob * 512:(ob + 1) * 512],
                                            start=(d == 0), stop=(d == ND - 1))
                                    nc.vector.tensor_copy(
                                        v_tiles[jt][:, ob * 512:(ob + 1) * 512], pv[:]
                                    )

            # ---------------- Phase B: attention ----------------
            with (
                tc.tile_pool(name="kc", bufs=6) as kcpool,
                tc.tile_pool(name="qc", bufs=2) as qcpool,
                tc.tile_pool(name="ex", bufs=4) as expool,
                tc.tile_pool(name="ost", bufs=4) as ostpool,
                tc.tile_pool(name="rcp", bufs=4) as rcpool,
                tc.tile_pool(name="psS", bufs=2, space="PSUM") as psS,
                tc.tile_pool(name="psC", bufs=1, space="PSUM") as psC,
                tc.tile_pool(name="psD", bufs=1, space="PSUM") as psD,
            ):
                for s in reversed(range(N_IB)):
                    jt_n = JT_SLOTS[s]
                    qc = qcpool.tile([P, NO, IB], F32R, tag="qc", name=f"qc{s}")
                    nc.gpsimd.dma_start(
                        qc[:], d_major(qT_dram[:, s * IB:(s + 1) * IB])
                    )
                    cps = [
                        [
                            psC.tile([P, 512], F32, tag=f"c{it}{ob}", name=f"c{s}_{it}{ob}")
                            for ob in range(2)
                        ]
                        for it in range(2)
                    ]
                    dps = [
                        psD.tile([P, 2], F32, tag=f"d{it}", name=f"d{s}_{it}")
                        for it in range(2)
                    ]
                    for t in range(jt_n):
                        if t >= 8:
                            kc = kcpool.tile([P, NO, P], F32R, tag="kc", name=f"kc{s}_{t}")
                            nc.scalar.dma_start(
                                kc[:], d_major(kT_dram[:, (t - 8) * P:(t - 7) * P])
                            )
                        ps = psS.tile([P, IB], F32, tag="ps", name=f"ps{s}_{t}")
                        for o in range(NO):
                            lhsk = (kc[:, o, :] if t >= 8
                                    else kT_res[o][:, t * P:(t + 1) * P])
                            _mm(nc, ps[:], lhsk, qc[:, o, :],
                                start=(o == 0), stop=(o == NO - 1))
                        et = expool.tile([P, IB], F32R, tag="et", name=f"et{s}_{t}")
                        if t >= jt_n - 4:
                            eraw = expool.tile([P, IB], F32R, tag="eraw", name=f"er{s}_{t}")
                            nc.scalar.activation(
                                eraw[:], ps[:],
                                mybir.ActivationFunctionType.Exp, scale=scale,
                            )
                            col = s * 4 + (t - (jt_n - 4))
                            nc.vector.scalar_tensor_tensor(
                                et[:], t0_t[:], delta_t[:, col:col + 1], eraw[:],
                                op0=mybir.AluOpType.is_le,
                                op1=mybir.AluOpType.mult,
                            )
                        else:
                            nc.scalar.activation(
                                et[:], ps[:],
                                mybir.ActivationFunctionType.Exp, scale=scale,
                            )
                        last = t == jt_n - 1
                        for it in range(2):
                            lhs = et[:, it * P:(it + 1) * P]
                            for ob in range(2):
                                _mm(nc, cps[it][ob][:], lhs,
                                    v_tiles[t][:, ob * 512:(ob + 1) * 512],
                                    start=(t == 0), stop=last)
                            _mm(nc, dps[it][:], lhs, ones_t[:],
                                start=(t == 0), stop=last)
                    for it in range(2):
                        rc = rcpool.tile([P, 1], F32, tag="rc", name=f"rc{s}_{it}")
                        nc.vector.reciprocal(rc[:], dps[it][:, 0:1])
                        ot = ostpool.tile([P, D], F32, tag="ot", name=f"ot{s}_{it}")
                        for ob in range(2):
                            nc.vector.tensor_scalar_mul(
                                ot[:, ob * 512:(ob + 1) * 512], cps[it][ob][:], rc[:]
                            )
                        nc.sync.dma_start(
                            out[s * IB + it * P: s * IB + (it + 1) * P, :], ot[:]
                        )

    nc.compile()
    return nc


_NC_CACHE = None


def _get_nc():
    global _NC_CACHE
    if _NC_CACHE is None:
        _NC_CACHE = build_program()
    return _NC_CACHE


def make_core_inputs(x, Wq, Wk, Wv):
    """Host-side shard prep. Returns list of 8 in_maps."""
    x = np.asarray(x, dtype=np.float32)
    wqT = np.ascontiguousarray(np.asarray(Wq, np.float32).T)
    wkT = np.ascontiguousarray(np.asarray(Wk, np.float32).T)
    wvT = np.ascontiguousarray(np.asarray(Wv, np.float32).T)
    t0 = (np.arange(P, dtype=np.float32)[:, None]
          - np.arange(IB, dtype=np.float32)[None, :])
    t0 = np.ascontiguousarray(t0)

    in_maps = []
    for c in range(N_CORES):
        b, r = divmod(c, 2)
        starts = ROLE_STARTS[r]
        xT = np.ascontiguousarray(x[b].T)
        xq = np.concatenate([x[b][i0:i0 + IB, :] for i0 in starts], axis=0)
        xqT = np.ascontiguousarray(xq.T)
        delta = np.empty((P, 16), np.float32)
        for s in range(N_IB):
            for tr in range(4):
                t = JT_SLOTS[s] - 4 + tr
                delta[:, s * 4 + tr] = float(starts[s] - P * t)
        in_maps.append({
            "xT": xT, "xqT": xqT,
            "wqT": wqT, "wkT": wkT, "wvT": wvT,
            "t0": t0, "delta": np.ascontiguousarray(delta),
            "ones": np.ones((P, 2), np.float32),
        })
    return in_maps


def assemble_output(results):
    """Gather 8 per-core [1024, 1024] outputs into [B, S, D]."""
    out = np.empty((B, S, D), np.float32)
    for c in range(N_CORES):
        b, r = divmod(c, 2)
        starts = ROLE_STARTS[r]
        oc = results[c]["out"]
        for s, i0 in enumerate(starts):
            out[b, i0:i0 + IB, :] = oc[s * IB:(s + 1) * IB, :]
    return out


def kernel(x, Wq, Wk, Wv):
    nc = _get_nc()
    in_maps = make_core_inputs(x, Wq, Wk, Wv)
    res = run_bass_kernel_spmd(nc, in_maps, list(range(N_CORES)))
    return assemble_output(res.results)



# revision 4
# speedup vs baseline: 1.5027x; 1.5027x over previous
"""
Causal self-attention (single head) on 8 trn2 NeuronCores.

Problem: x[4, 2048, 1024], Wq/Wk/Wv[1024, 1024] (torch Linear layout
[d_out, d_in]).
    q/k/v = x @ W.T ; out = softmax(mask(q k^T) / 32) @ v

Sharding v2 — flash-style key split (no collectives, uniform SPMD
program, role differences live entirely in the INPUTS):
  core c -> batch b = c // 2, role r = c % 2.
  Keys/values are split between the pair by alternating 128-row
  j-tiles: core r owns global j-tiles {2t + r}.  Each core projects
  K/V only for its own 1024 key rows (no duplicated K/V work), but
  projects Q for all 2048 query rows (duplicated — the only
  duplicated matmul work).
  Each core computes partial ctx = sum_j exp(s_j) v_j and partial
  den = sum_j exp(s_j) over ITS j-tiles for ALL queries; the host
  combines: out = (ctxE + ctxO) / (denE + denO).  exp uses no
  running-max (logits/32 are O(2), exp is safe in fp32/bf16).

  Causal structure: query i-block ib (256 rows) needs own-j-tiles
  t = 0..ib on BOTH roles (perfect balance), and only the last tile
  t == ib is on the diagonal and needs masking.  The mask is a
  CONSTANT per role: keep jj - ii <= 0 (r=0) / jj - ii <= -128 (r=1),
  uploaded as a bf16 0/1 tile and multiplied after exp.

All matmuls run in bf16 (1 cycle/row on the PE -- measured ~0.43
ns/row at N=512, LDWEIGHTS fully hidden); PSUM accumulation is fp32.
bf16 end-to-end keeps rel err ~5e-3 (simulated) vs the 2e-2 gate.
Host pre-transposes/casts x and W once per call; inputs are 10MB/core.

Layouts (on-chip matmuls contract over the partition dim):
  xT   [d 128, nd 8, s 2048]   bf16  (Q projection rhs)
  xTj  [d 128, nd 8, j 1024]   bf16  own key rows, pre-gathered host-side
  wqT/wkT/wvT [d 128, nd 8, o 1024] bf16
  qT   [o 128, no 8, i 2048]   bf16  <- psum copies
  kT   [o 128, no 8, j 1024]   bf16
  v    [j 128, t 8,  o 1024]   bf16
  scoresT psum [j 128, i 256] = kT-tile^T @ qT-chunk   (contract o)
  et   = exp(scoresT / 32)  (scalar ACT, bf16 out; diag: * mask)
  ctx  psum [i 128, o 512] += et-tile^T @ v-tile       (contract j)
  den  psum [i 128, 2]     += et-tile^T @ ones
  out: ctx -> bf16 DMA, den -> fp32 DMA (division happens on host)
"""

import sys

for _p in ("/opt/trn_rl_repo", "/root/.axon_site/_ro/trn_rl_repo"):
    if _p not in sys.path:
        sys.path.append(_p)

import numpy as np
import ml_dtypes

import concourse.bass as bass
import concourse.mybir as mybir
import concourse.tile as tile
from concourse import bacc
from concourse.bass_utils import run_bass_kernel_spmd

F32 = mybir.dt.float32
BF16 = mybir.dt.bfloat16
NPBF16 = ml_dtypes.bfloat16

B, S, D = 4, 2048, 1024
P = 128
ND = D // P          # 8 d-tiles (projection contraction)
NO = D // P          # 8 o-tiles
IB = 256             # query block rows
N_IB = S // IB       # 8 query blocks
JH = S // 2          # 1024 own key rows per core
NJT = JH // P        # 8 own j-tiles
N_CORES = 8


def _mm(nc, out, lhsT, rhs, start, stop):
    nc.tensor.matmul(out, lhsT, rhs, start=start, stop=stop)


def build_program():
    nc = bacc.Bacc(
        "TRN2",
        target_bir_lowering=False,
        debug=False,
        enable_asserts=False,
        num_devices=N_CORES,
    )
    xT = nc.dram_tensor("xT", [D, S], BF16, kind="ExternalInput").ap()
    xTj = nc.dram_tensor("xTj", [D, JH], BF16, kind="ExternalInput").ap()
    wqT = nc.dram_tensor("wqT", [D, D], BF16, kind="ExternalInput").ap()
    wkT = nc.dram_tensor("wkT", [D, D], BF16, kind="ExternalInput").ap()
    wvT = nc.dram_tensor("wvT", [D, D], BF16, kind="ExternalInput").ap()
    mask_in = nc.dram_tensor("mask", [P, IB], BF16, kind="ExternalInput").ap()
    ones_in = nc.dram_tensor("ones", [P, 2], BF16, kind="ExternalInput").ap()
    ctx_out = nc.dram_tensor("ctx", [S, D], BF16, kind="ExternalOutput").ap()
    den_out = nc.dram_tensor("den", [S, 2], F32, kind="ExternalOutput").ap()

    scale = 1.0 / 32.0  # 1/sqrt(d_v)

    def d_major(ap2d):
        # [ND*P, C] DRAM view -> [P, ND, C] (partition-major 3D AP)
        return ap2d.rearrange("(nd p) c -> p nd c", p=P)

    with tile.TileContext(nc) as tc:
        with (
            tc.tile_pool(name="const", bufs=1) as cpool,
            tc.tile_pool(name="res", bufs=1) as rpool,
        ):
            mask_t = cpool.tile([P, IB], BF16, tag="mask")
            nc.sync.dma_start(mask_t[:], mask_in[:])
            ones_t = cpool.tile([P, 2], BF16, tag="ones")
            nc.sync.dma_start(ones_t[:], ones_in[:])

            qT = rpool.tile([P, NO, S], BF16, tag="qT", name="qT")
            kT = rpool.tile([P, NO, JH], BF16, tag="kT", name="kT")
            v_t = rpool.tile([P, NJT, D], BF16, tag="v", name="v")

            # ---------------- Phase A: projections ----------------
            with (
                tc.tile_pool(name="xp", bufs=1) as xpool,
                tc.tile_pool(name="wp", bufs=1) as wpool,
                tc.tile_pool(name="psA", bufs=3, space="PSUM") as psA,
            ):
                wq_t = wpool.tile([P, ND, D], BF16, tag="wq", name="wq")
                nc.sync.dma_start(wq_t[:, 0:4, :], d_major(wqT[0:4 * P, :]))
                nc.gpsimd.dma_start(wq_t[:, 4:8, :], d_major(wqT[4 * P:, :]))
                xT_t = xpool.tile([P, ND, S], BF16, tag="xT", name="xT")
                nc.scalar.dma_start(
                    xT_t[:, :, 0:512], d_major(xT[:, 0:512])
                )

                # PE warm-up to lift the clock gate while loads land.
                warm = xpool.tile([P, 512], BF16, tag="warm", name="warm")
                nc.sync.dma_start(warm[:], xT[0:P, 0:512])
                wps = psA.tile([P, 512], F32, tag="wps", name="wps", bufs=1)
                for w in range(48):
                    _mm(nc, wps[:], warm[:, 0:P], warm[:], start=True, stop=True)

                nc.scalar.dma_start(
                    xT_t[:, :, 512:2048], d_major(xT[:, 512:2048])
                )
                xTj_t = xpool.tile([P, ND, JH], BF16, tag="xTj", name="xTj")
                nc.gpsimd.dma_start(xTj_t[:], d_major(xTj))
                wk_t = wpool.tile([P, ND, D], BF16, tag="wk", name="wk")
                nc.gpsimd.dma_start(wk_t[:], d_major(wkT))
                wv_t = wpool.tile([P, ND, D], BF16, tag="wv", name="wv")
                nc.scalar.dma_start(wv_t[:], d_major(wvT))

                # --- Q projection: qT[o, :, i] ---
                for ic in range(S // 512):
                    for o in range(NO):
                        pq = psA.tile([P, 512], F32, tag="pp", name=f"pq{ic}_{o}")
                        for d in range(ND):
                            _mm(nc, pq[:],
                                wq_t[:, d, o * P:(o + 1) * P],
                                xT_t[:, d, ic * 512:(ic + 1) * 512],
                                start=(d == 0), stop=(d == ND - 1))
                        if o % 2 == 0:
                            nc.vector.tensor_copy(
                                qT[:, o, ic * 512:(ic + 1) * 512], pq[:])
                        else:
                            nc.scalar.copy(
                                qT[:, o, ic * 512:(ic + 1) * 512], pq[:])

                # --- K projection: kT[o, :, j_local] ---
                for jc in range(JH // 512):
                    for o in range(NO):
                        pk = psA.tile([P, 512], F32, tag="pp", name=f"pk{jc}_{o}")
                        for d in range(ND):
                            _mm(nc, pk[:],
                                wk_t[:, d, o * P:(o + 1) * P],
                                xTj_t[:, d, jc * 512:(jc + 1) * 512],
                                start=(d == 0), stop=(d == ND - 1))
                        if o % 2 == 0:
                            nc.vector.tensor_copy(
                                kT[:, o, jc * 512:(jc + 1) * 512], pk[:])
                        else:
                            nc.scalar.copy(
                                kT[:, o, jc * 512:(jc + 1) * 512], pk[:])

                # --- V projection: v[j 128, t, o] ---
                for t in range(NJT):
                    for ob in range(2):
                        pv = psA.tile([P, 512], F32, tag="pp", name=f"pv{t}_{ob}")
                        for d in range(ND):
                            _mm(nc, pv[:],
                                xTj_t[:, d, t * P:(t + 1) * P],
                                wv_t[:, d, ob * 512:(ob + 1) * 512],
                                start=(d == 0), stop=(d == ND - 1))
                        if ob % 2 == 0:
                            nc.vector.tensor_copy(
                                v_t[:, t, ob * 512:(ob + 1) * 512], pv[:])
                        else:
                            nc.scalar.copy(
                                v_t[:, t, ob * 512:(ob + 1) * 512], pv[:])

            # ---------------- Phase B: attention ----------------
            with (
                tc.tile_pool(name="ex", bufs=4) as expool,
                tc.tile_pool(name="md", bufs=2) as mdpool,
                tc.tile_pool(name="ost", bufs=4) as ostpool,
                tc.tile_pool(name="dst", bufs=4) as dstpool,
                tc.tile_pool(name="psS", bufs=2, space="PSUM") as psS,
                tc.tile_pool(name="psC", bufs=1, space="PSUM") as psC,
                tc.tile_pool(name="psD", bufs=1, space="PSUM") as psD,
            ):
                for ib in range(N_IB):
                    njt = ib + 1
                    cps = [
                        [
                            psC.tile([P, 512], F32, tag=f"c{it}{ob}",
                                     name=f"c{ib}_{it}{ob}")
                            for ob in range(2)
                        ]
                        for it in range(2)
                    ]
                    dps = [
                        psD.tile([P, 2], F32, tag=f"d{it}", name=f"d{ib}_{it}")
                        for it in range(2)
                    ]
                    for t in range(njt):
                        ps = psS.tile([P, IB], F32, tag="ps", name=f"ps{ib}_{t}")
                        for o in range(NO):
                            _mm(nc, ps[:],
                                kT[:, o, t * P:(t + 1) * P],
                                qT[:, o, ib * IB:(ib + 1) * IB],
                                start=(o == 0), stop=(o == NO - 1))
                        if t == njt - 1:
                            eraw = expool.tile([P, IB], BF16, tag="et",
                                               name=f"er{ib}_{t}")
                            nc.scalar.activation(
                                eraw[:], ps[:],
                                mybir.ActivationFunctionType.Exp, scale=scale,
                            )
                            et = mdpool.tile([P, IB], BF16, tag="md",
                                             name=f"md{ib}")
                            nc.vector.tensor_mul(et[:], eraw[:], mask_t[:])
                        else:
                            et = expool.tile([P, IB], BF16, tag="et",
                                             name=f"et{ib}_{t}")
                            nc.scalar.activation(
                                et[:], ps[:],
                                mybir.ActivationFunctionType.Exp, scale=scale,
                            )
                        last = t == njt - 1
                        for it in range(2):
                            lhs = et[:, it * P:(it + 1) * P]
                            for ob in range(2):
                                _mm(nc, cps[it][ob][:], lhs,
                                    v_t[:, t, ob * 512:(ob + 1) * 512],
                                    start=(t == 0), stop=last)
                            _mm(nc, dps[it][:], lhs, ones_t[:],
                                start=(t == 0), stop=last)
                    for it in range(2):
                        row0 = ib * IB + it * P
                        ds = dstpool.tile([P, 2], F32, tag="ds",
                                          name=f"ds{ib}_{it}")
                        nc.vector.tensor_copy(ds[:], dps[it][:])
                        nc.scalar.dma_start(den_out[row0:row0 + P, :], ds[:])
                        ot = ostpool.tile([P, D], BF16, tag="ot",
                                          name=f"ot{ib}_{it}")
                        nc.vector.tensor_copy(ot[:, 0:512], cps[it][0][:])
                        nc.scalar.copy(ot[:, 512:1024], cps[it][1][:])
                        nc.sync.dma_start(ctx_out[row0:row0 + P, :], ot[:])

    nc.compile()
    return nc


_NC_CACHE = None


def _get_nc():
    global _NC_CACHE
    if _NC_CACHE is None:
        _NC_CACHE = build_program()
    return _NC_CACHE


def make_core_inputs(x, Wq, Wk, Wv):
    """Host-side shard prep. Returns list of 8 in_maps."""
    x = np.asarray(x, dtype=np.float32)
    wqT = np.ascontiguousarray(np.asarray(Wq, np.float32).T).astype(NPBF16)
    wkT = np.ascontiguousarray(np.asarray(Wk, np.float32).T).astype(NPBF16)
    wvT = np.ascontiguousarray(np.asarray(Wv, np.float32).T).astype(NPBF16)
    ones = np.ones((P, 2), NPBF16)

    # mask[jj, ii] for the diagonal j-tile: keep jj - ii <= delta
    jj = np.arange(P, dtype=np.float32)[:, None]
    ii = np.arange(IB, dtype=np.float32)[None, :]
    masks = [
        ((jj - ii) <= delta).astype(NPBF16) for delta in (0.0, -128.0)
    ]

    in_maps = []
    for c in range(N_CORES):
        b, r = divmod(c, 2)
        xb = x[b]                          # [S, D] fp32
        xT = np.ascontiguousarray(xb.T).astype(NPBF16)       # [D, S]
        # own key rows: global j-tiles 2t + r -> rows [ (2t+r)*128, +128 )
        rows = np.concatenate(
            [np.arange((2 * t + r) * P, (2 * t + r + 1) * P) for t in range(NJT)]
        )
        xTj = np.ascontiguousarray(xb[rows, :].T).astype(NPBF16)  # [D, JH]
        in_maps.append({
            "xT": xT, "xTj": xTj,
            "wqT": wqT, "wkT": wkT, "wvT": wvT,
            "mask": masks[r], "ones": ones,
        })
    return in_maps


def assemble_output(results):
    """Combine per-core partial (ctx, den) into the full [B, S, D] output."""
    out = np.empty((B, S, D), np.float32)
    for b in range(B):
        ctx_e = np.asarray(results[2 * b]["ctx"]).astype(np.float32)
        ctx_o = np.asarray(results[2 * b + 1]["ctx"]).astype(np.float32)
        den_e = np.asarray(results[2 * b]["den"])[:, 0:1]
        den_o = np.asarray(results[2 * b + 1]["den"])[:, 0:1]
        out[b] = (ctx_e + ctx_o) / (den_e + den_o)
    return out


def kernel(x, Wq, Wk, Wv):
    nc = _get_nc()
    in_maps = make_core_inputs(x, Wq, Wk, Wv)
    res = run_bass_kernel_spmd(nc, in_maps, list(range(N_CORES)))
    return assemble_output(res.results)


# revision 7
# speedup vs baseline: 1.5676x; 1.0431x over previous
"""
Causal self-attention (single head) on 8 trn2 NeuronCores.

Problem: x[4, 2048, 1024], Wq/Wk/Wv[1024, 1024] (torch Linear layout
[d_out, d_in]).
    q/k/v = x @ W.T ; out = softmax(mask(q k^T) / 32) @ v

Sharding — flash-style key split (no collectives, uniform SPMD
program, role differences live entirely in the INPUTS):
  core c -> batch b = c // 2, role r = c % 2.
  Keys/values are split between the pair by alternating 128-row
  j-tiles: core r owns global j-tiles {2t + r}.  Each core projects
  K/V only for its own 1024 key rows (no duplicated K/V work), but
  projects Q for all 2048 query rows (the only duplicated work).
  Each core computes partial ctx = sum_j exp(s_j) v_j and partial
  den = sum_j exp(s_j) over ITS j-tiles for ALL queries; the host
  combines: out = (ctxE + ctxO) / (denE + denO).  exp needs no
  running-max (logits/32 are O(2), exp is safe in fp32/bf16).

  Causal structure: query i-block ib (256 rows) needs own-j-tiles
  t = 0..ib on BOTH roles (perfect balance); only the diagonal tile
  t == ib needs masking, and the mask is CONSTANT per role:
  keep jj - ii <= 0 (r=0) / <= -128 (r=1), uploaded as bf16 0/1.

All matmuls are bf16 (measured ~0.43 ns/row at N=512, ~1 cyc/row,
LDWEIGHTS hidden); PSUM accumulates fp32.  End-to-end rel err ~5e-3
vs the 2e-2 gate.  fp8 was measured at 1.8-2.1e-2 — too close.

Schedule notes (from perfetto traces):
 - Input DMAs are issued in priority order: wq spread over all 4 DMA
   queues + first xT chunks first (the 11 MB of input at once made the
   PE idle ~23 us at start); bulk loads are issued progressively.
 - Attention inner loop is software-pipelined: AV(t-1) is issued
   AFTER scores(t), so the exp(t-1) on the scalar engine has the whole
   scores(t) window to complete (otherwise the AV LDWEIGHTS stalls
   ~0.5-2 us per tile on the exp semaphore).
 - In attention the scalar engine does ONLY exp; ctx/den PSUM copies
   go to vector; output DMA issue is spread sync/gpsimd.
"""

import sys

for _p in ("/opt/trn_rl_repo", "/root/.axon_site/_ro/trn_rl_repo"):
    if _p not in sys.path:
        sys.path.append(_p)

import numpy as np
import ml_dtypes

import concourse.bass as bass
import concourse.mybir as mybir
import concourse.tile as tile
from concourse import bacc
from concourse.bass_utils import run_bass_kernel_spmd

F32 = mybir.dt.float32
BF16 = mybir.dt.bfloat16
NPBF16 = ml_dtypes.bfloat16

B, S, D = 4, 2048, 1024
P = 128
ND = D // P          # 8 d-tiles (projection contraction)
NO = D // P          # 8 o-tiles
IB = 256             # query block rows
N_IB = S // IB       # 8 query blocks
JH = S // 2          # 1024 own key rows per core
NJT = JH // P        # 8 own j-tiles
N_CORES = 8


def _mm(nc, out, lhsT, rhs, start, stop):
    nc.tensor.matmul(out, lhsT, rhs, start=start, stop=stop)


def build_program():
    nc = bacc.Bacc(
        "TRN2",
        target_bir_lowering=False,
        debug=False,
        enable_asserts=False,
        num_devices=N_CORES,
    )
    xT = nc.dram_tensor("xT", [D, S], BF16, kind="ExternalInput").ap()
    xTj = nc.dram_tensor("xTj", [D, JH], BF16, kind="ExternalInput").ap()
    wqT = nc.dram_tensor("wqT", [D, D], BF16, kind="ExternalInput").ap()
    wkT = nc.dram_tensor("wkT", [D, D], BF16, kind="ExternalInput").ap()
    wvT = nc.dram_tensor("wvT", [D, D], BF16, kind="ExternalInput").ap()
    mask_in = nc.dram_tensor("mask", [P, IB], BF16, kind="ExternalInput").ap()
    ones_in = nc.dram_tensor("ones", [P, 2], BF16, kind="ExternalInput").ap()
    ctx_out = nc.dram_tensor("ctx", [S, D], BF16, kind="ExternalOutput").ap()
    den_out = nc.dram_tensor("den", [S, 2], F32, kind="ExternalOutput").ap()

    scale = 1.0 / 32.0  # 1/sqrt(d_v)

    def d_major(ap2d):
        # [ND*P, C] DRAM view -> [P, ND, C] (partition-major 3D AP)
        return ap2d.rearrange("(nd p) c -> p nd c", p=P)

    with tile.TileContext(nc) as tc:
        with (
            tc.tile_pool(name="const", bufs=1) as cpool,
            tc.tile_pool(name="res", bufs=1) as rpool,
        ):
            mask_t = cpool.tile([P, IB], BF16, tag="mask")
            nc.sync.dma_start(mask_t[:], mask_in[:])
            ones_t = cpool.tile([P, 2], BF16, tag="ones")
            nc.sync.dma_start(ones_t[:], ones_in[:])

            qT = rpool.tile([P, NO, S], BF16, tag="qT", name="qT")
            kT = rpool.tile([P, NO, JH], BF16, tag="kT", name="kT")
            v_t = rpool.tile([P, NJT, D], BF16, tag="v", name="v")

            # ---------------- Phase A: projections ----------------
            with (
                tc.tile_pool(name="xp", bufs=1) as xpool,
                tc.tile_pool(name="wp", bufs=1) as wpool,
                tc.tile_pool(name="psA", bufs=3, space="PSUM") as psA,
            ):
                # Priority loads: wq striped over all 4 DMA queues, then
                # the first two xT chunks.  Everything else is issued
                # progressively below so it doesn't steal HBM bandwidth
                # from the critical path.
                wq_t = wpool.tile([P, ND, D], BF16, tag="wq", name="wq")
                nc.sync.dma_start(wq_t[:, 0:3, :], d_major(wqT[0:3 * P, :]))
                nc.gpsimd.dma_start(wq_t[:, 3:6, :], d_major(wqT[3 * P:6 * P, :]))
                nc.scalar.dma_start(wq_t[:, 6:8, :], d_major(wqT[6 * P:, :]))
                xT_t = xpool.tile([P, ND, S], BF16, tag="xT", name="xT")
                nc.scalar.dma_start(xT_t[:, :, 0:512], d_major(xT[:, 0:512]))
                nc.gpsimd.dma_start(
                    xT_t[:, :, 512:1024], d_major(xT[:, 512:1024])
                )

                # PE warm-up to lift the clock gate while loads land.
                warm = xpool.tile([P, 512], BF16, tag="warm", name="warm")
                nc.sync.dma_start(warm[:], xT[0:P, 0:512])
                wps = psA.tile([P, 512], F32, tag="wps", name="wps", bufs=1)
                for w in range(48):
                    _mm(nc, wps[:], warm[:, 0:P], warm[:], start=True, stop=True)

                xTj_t = xpool.tile([P, ND, JH], BF16, tag="xTj", name="xTj")
                wk_t = wpool.tile([P, ND, D], BF16, tag="wk", name="wk")
                wv_t = wpool.tile([P, ND, D], BF16, tag="wv", name="wv")

                # --- Q projection: qT[o, :, i] ---
                for ic in range(S // 512):
                    for o in range(NO):
                        pq = psA.tile([P, 512], F32, tag="pp", name=f"pq{ic}_{o}")
                        for d in range(ND):
                            _mm(nc, pq[:],
                                wq_t[:, d, o * P:(o + 1) * P],
                                xT_t[:, d, ic * 512:(ic + 1) * 512],
                                start=(d == 0), stop=(d == ND - 1))
                        if o % 2 == 0:
                            nc.vector.tensor_copy(
                                qT[:, o, ic * 512:(ic + 1) * 512], pq[:])
                        else:
                            nc.scalar.copy(
                                qT[:, o, ic * 512:(ic + 1) * 512], pq[:])
                    # progressive bulk loads, issued behind the compute
                    if ic == 0:
                        nc.sync.dma_start(
                            xT_t[:, :, 1024:1536], d_major(xT[:, 1024:1536])
                        )
                    elif ic == 1:
                        nc.gpsimd.dma_start(
                            xT_t[:, :, 1536:2048], d_major(xT[:, 1536:2048])
                        )
                    elif ic == 2:
                        nc.gpsimd.dma_start(xTj_t[:], d_major(xTj))
                        nc.scalar.dma_start(wk_t[:], d_major(wkT))

                # --- K projection: kT[o, :, j_local] ---
                for jc in range(JH // 512):
                    for o in range(NO):
                        pk = psA.tile([P, 512], F32, tag="pp", name=f"pk{jc}_{o}")
                        for d in range(ND):
                            _mm(nc, pk[:],
                                wk_t[:, d, o * P:(o + 1) * P],
                                xTj_t[:, d, jc * 512:(jc + 1) * 512],
                                start=(d == 0), stop=(d == ND - 1))
                        if o % 2 == 0:
                            nc.vector.tensor_copy(
                                kT[:, o, jc * 512:(jc + 1) * 512], pk[:])
                        else:
                            nc.scalar.copy(
                                kT[:, o, jc * 512:(jc + 1) * 512], pk[:])
                    if jc == 0:
                        nc.gpsimd.dma_start(wv_t[:], d_major(wvT))

                # --- V projection: v[j 128, t, o] ---
                for t in range(NJT):
                    for ob in range(2):
                        pv = psA.tile([P, 512], F32, tag="pp", name=f"pv{t}_{ob}")
                        for d in range(ND):
                            _mm(nc, pv[:],
                                xTj_t[:, d, t * P:(t + 1) * P],
                                wv_t[:, d, ob * 512:(ob + 1) * 512],
                                start=(d == 0), stop=(d == ND - 1))
                        if ob % 2 == 0:
                            nc.vector.tensor_copy(
                                v_t[:, t, ob * 512:(ob + 1) * 512], pv[:])
                        else:
                            nc.scalar.copy(
                                v_t[:, t, ob * 512:(ob + 1) * 512], pv[:])

            # ---------------- Phase B: attention ----------------
            with (
                tc.tile_pool(name="ex", bufs=4) as expool,
                tc.tile_pool(name="ost", bufs=4) as ostpool,
                tc.tile_pool(name="dst", bufs=4) as dstpool,
                tc.tile_pool(name="psS", bufs=2, space="PSUM") as psS,
                tc.tile_pool(name="psC", bufs=1, space="PSUM") as psC,
                tc.tile_pool(name="psD", bufs=1, space="PSUM") as psD,
            ):
                for ib in range(N_IB):
                    njt = ib + 1
                    cps = [
                        [
                            psC.tile([P, 512], F32, tag=f"c{it}{ob}",
                                     name=f"c{ib}_{it}{ob}")
                            for ob in range(2)
                        ]
                        for it in range(2)
                    ]
                    dps = [
                        psD.tile([P, 2], F32, tag=f"d{it}", name=f"d{ib}_{it}")
                        for it in range(2)
                    ]

                    def issue_av(t, et):
                        last = t == njt - 1
                        for it in range(2):
                            lhs = et[:, it * P:(it + 1) * P]
                            for ob in range(2):
                                _mm(nc, cps[it][ob][:], lhs,
                                    v_t[:, t, ob * 512:(ob + 1) * 512],
                                    start=(t == 0), stop=last)
                            _mm(nc, dps[it][:], lhs, ones_t[:],
                                start=(t == 0), stop=last)

                    # software pipeline: AV(t-1) issued after scores(t),
                    # giving exp(t-1) the scores(t) window to complete
                    et_prev = None
                    for t in range(njt):
                        ps = psS.tile([P, IB], F32, tag="ps", name=f"ps{ib}_{t}")
                        for o in range(NO):
                            _mm(nc, ps[:],
                                kT[:, o, t * P:(t + 1) * P],
                                qT[:, o, ib * IB:(ib + 1) * IB],
                                start=(o == 0), stop=(o == NO - 1))
                        et = expool.tile([P, IB], BF16, tag="et",
                                         name=f"et{ib}_{t}")
                        nc.scalar.activation(
                            et[:], ps[:],
                            mybir.ActivationFunctionType.Exp, scale=scale,
                        )
                        if t == njt - 1:
                            etm = expool.tile([P, IB], BF16, tag="md",
                                              name=f"md{ib}")
                            nc.vector.tensor_mul(etm[:], et[:], mask_t[:])
                            et = etm
                        if et_prev is not None:
                            issue_av(t - 1, et_prev)
                        et_prev = et
                    issue_av(njt - 1, et_prev)

                    for it in range(2):
                        row0 = ib * IB + it * P
                        ds = dstpool.tile([P, 2], F32, tag="ds",
                                          name=f"ds{ib}_{it}")
                        nc.vector.tensor_copy(ds[:], dps[it][:])
                        nc.gpsimd.dma_start(den_out[row0:row0 + P, :], ds[:])
                        ot = ostpool.tile([P, D], BF16, tag="ot",
                                          name=f"ot{ib}_{it}")
                        nc.vector.tensor_copy(ot[:, 0:512], cps[it][0][:])
                        nc.vector.tensor_copy(ot[:, 512:1024], cps[it][1][:])
                        eng = nc.sync if it == 0 else nc.gpsimd
                        eng.dma_start(ctx_out[row0:row0 + P, :], ot[:])

    nc.compile()
    return nc


_NC_CACHE = None


def _get_nc():
    global _NC_CACHE
    if _NC_CACHE is None:
        _NC_CACHE = build_program()
    return _NC_CACHE


def make_core_inputs(x, Wq, Wk, Wv):
    """Host-side shard prep. Returns list of 8 in_maps."""
    x = np.asarray(x, dtype=np.float32)
    wqT = np.ascontiguousarray(np.asarray(Wq, np.float32).T).astype(NPBF16)
    wkT = np.ascontiguousarray(np.asarray(Wk, np.float32).T).astype(NPBF16)
    wvT = np.ascontiguousarray(np.asarray(Wv, np.float32).T).astype(NPBF16)
    ones = np.ones((P, 2), NPBF16)

    # mask[jj, ii] for the diagonal j-tile: keep jj - ii <= delta
    jj = np.arange(P, dtype=np.float32)[:, None]
    ii = np.arange(IB, dtype=np.float32)[None, :]
    masks = [
        ((jj - ii) <= delta).astype(NPBF16) for delta in (0.0, -128.0)
    ]

    in_maps = []
    for c in range(N_CORES):
        b, r = divmod(c, 2)
        xb = x[b]                          # [S, D] fp32
        xT = np.ascontiguousarray(xb.T).astype(NPBF16)       # [D, S]
        # own key rows: global j-tiles 2t + r -> rows [ (2t+r)*128, +128 )
        rows = np.concatenate(
            [np.arange((2 * t + r) * P, (2 * t + r + 1) * P) for t in range(NJT)]
        )
        xTj = np.ascontiguousarray(xb[rows, :].T).astype(NPBF16)  # [D, JH]
        in_maps.append({
            "xT": xT, "xTj": xTj,
            "wqT": wqT, "wkT": wkT, "wvT": wvT,
            "mask": masks[r], "ones": ones,
        })
    return in_maps


def assemble_output(results):
    """Combine per-core partial (ctx, den) into the full [B, S, D] output."""
    out = np.empty((B, S, D), np.float32)
    for b in range(B):
        ctx_e = np.asarray(results[2 * b]["ctx"]).astype(np.float32)
        ctx_o = np.asarray(results[2 * b + 1]["ctx"]).astype(np.float32)
        den_e = np.asarray(results[2 * b]["den"])[:, 0:1]
        den_o = np.asarray(results[2 * b + 1]["den"])[:, 0:1]
        out[b] = (ctx_e + ctx_o) / (den_e + den_o)
    return out


def kernel(x, Wq, Wk, Wv):
    nc = _get_nc()
    in_maps = make_core_inputs(x, Wq, Wk, Wv)
    res = run_bass_kernel_spmd(nc, in_maps, list(range(N_CORES)))
    return assemble_output(res.results)


# revision 11
# speedup vs baseline: 1.5787x; 1.0071x over previous
"""
Causal self-attention (single head) on 8 trn2 NeuronCores.

Problem: x[4, 2048, 1024], Wq/Wk/Wv[1024, 1024] (torch Linear layout
[d_out, d_in]).
    q/k/v = x @ W.T ; out = softmax(mask(q k^T) / 32) @ v

Sharding — flash-style key split (no collectives, uniform SPMD
program, role differences live entirely in the INPUTS):
  core c -> batch b = c // 2, role r = c % 2.
  Keys/values are split between the pair by alternating 128-row
  j-tiles: core r owns global j-tiles {2t + r}.  Each core projects
  K/V only for its own 1024 key rows (no duplicated K/V work), but
  projects Q for all 2048 query rows (the only duplicated work).
  Each core computes partial ctx = sum_j exp(s_j) v_j and partial
  den = sum_j exp(s_j) over ITS j-tiles for ALL queries; the host
  combines: out = (ctxE + ctxO) / (denE + denO).  exp needs no
  running-max (logits/32 are O(2), exp is safe in fp32/bf16).

  Causal structure: query i-block ib (256 rows) needs own-j-tiles
  t = 0..ib on BOTH roles (perfect balance); only the diagonal tile
  t == ib needs masking, and the mask is CONSTANT per role:
  keep jj - ii <= 0 (r=0) / <= -128 (r=1), uploaded as bf16 0/1.

All matmuls are bf16 (measured ~0.43 ns/row at N=512, ~1 cyc/row,
LDWEIGHTS hidden); PSUM accumulates fp32.  End-to-end rel err ~5e-3
vs the 2e-2 gate.  fp8 was measured at 1.8-2.1e-2 — too close.

Schedule notes (from perfetto traces):
 - Input DMAs are issued in priority order: wq spread over all 4 DMA
   queues + first xT chunks first (the 11 MB of input at once made the
   PE idle ~23 us at start); bulk loads are issued progressively.
 - Attention inner loop is software-pipelined: AV(t-1) is issued
   AFTER scores(t), so the exp(t-1) on the scalar engine has the whole
   scores(t) window to complete (otherwise the AV LDWEIGHTS stalls
   ~0.5-2 us per tile on the exp semaphore).
 - In attention the scalar engine does ONLY exp; ctx/den PSUM copies
   go to vector; output DMA issue is spread sync/gpsimd.
"""

import sys

for _p in ("/opt/trn_rl_repo", "/root/.axon_site/_ro/trn_rl_repo"):
    if _p not in sys.path:
        sys.path.append(_p)

import numpy as np
import ml_dtypes

import concourse.bass as bass
import concourse.mybir as mybir
import concourse.tile as tile
from concourse import bacc
from concourse.bass_utils import run_bass_kernel_spmd

F32 = mybir.dt.float32
BF16 = mybir.dt.bfloat16
NPBF16 = ml_dtypes.bfloat16

B, S, D = 4, 2048, 1024
P = 128
ND = D // P          # 8 d-tiles (projection contraction)
NO = D // P          # 8 o-tiles
IB = 256             # query block rows
N_IB = S // IB       # 8 query blocks
JH = S // 2          # 1024 own key rows per core
NJT = JH // P        # 8 own j-tiles
N_CORES = 8


def _mm(nc, out, lhsT, rhs, start, stop):
    nc.tensor.matmul(out, lhsT, rhs, start=start, stop=stop)


def build_program():
    nc = bacc.Bacc(
        "TRN2",
        target_bir_lowering=False,
        debug=False,
        enable_asserts=False,
        num_devices=N_CORES,
    )
    xT = nc.dram_tensor("xT", [D, S], BF16, kind="ExternalInput").ap()
    xTj = nc.dram_tensor("xTj", [D, JH], BF16, kind="ExternalInput").ap()
    wqT = nc.dram_tensor("wqT", [D, D], BF16, kind="ExternalInput").ap()
    wkT = nc.dram_tensor("wkT", [D, D], BF16, kind="ExternalInput").ap()
    wvT = nc.dram_tensor("wvT", [D, D], BF16, kind="ExternalInput").ap()
    mask_in = nc.dram_tensor("mask", [P, IB], BF16, kind="ExternalInput").ap()
    ones_in = nc.dram_tensor("ones", [P, 2], BF16, kind="ExternalInput").ap()
    ctx_out = nc.dram_tensor("ctx", [S, D], BF16, kind="ExternalOutput").ap()
    den_out = nc.dram_tensor("den", [S, 2], F32, kind="ExternalOutput").ap()

    scale = 1.0 / 32.0  # 1/sqrt(d_v)

    def d_major(ap2d):
        # [ND*P, C] DRAM view -> [P, ND, C] (partition-major 3D AP)
        return ap2d.rearrange("(nd p) c -> p nd c", p=P)

    with tile.TileContext(nc) as tc:
        with (
            tc.tile_pool(name="const", bufs=1) as cpool,
            tc.tile_pool(name="res", bufs=1) as rpool,
        ):
            # warm tile rides first on the sync queue so the PE can start
            # ramping as soon as the DMA engines spin up (~10 us).
            warm = cpool.tile([P, 512], BF16, tag="warm", name="warm")
            nc.sync.dma_start(warm[:], xT[0:P, 0:512])
            mask_t = cpool.tile([P, IB], BF16, tag="mask")
            nc.sync.dma_start(mask_t[:], mask_in[:])
            ones_t = cpool.tile([P, 2], BF16, tag="ones")
            nc.sync.dma_start(ones_t[:], ones_in[:])

            qT = rpool.tile([P, NO, S], BF16, tag="qT", name="qT")
            kT = rpool.tile([P, NO, JH], BF16, tag="kT", name="kT")
            v_t = rpool.tile([P, NJT, D], BF16, tag="v", name="v")

            # ---------------- Phase A: projections ----------------
            with (
                tc.tile_pool(name="xp", bufs=1) as xpool,
                tc.tile_pool(name="wp", bufs=1) as wpool,
                tc.tile_pool(name="psA", bufs=3, space="PSUM") as psA,
            ):
                # Priority loads: wq striped over all 4 DMA queues, then
                # the first two xT chunks.  Everything else is issued
                # progressively below so it doesn't steal HBM bandwidth
                # from the critical path.
                wq_t = wpool.tile([P, ND, D], BF16, tag="wq", name="wq")
                nc.sync.dma_start(wq_t[:, 0:3, :], d_major(wqT[0:3 * P, :]))
                nc.gpsimd.dma_start(wq_t[:, 3:6, :], d_major(wqT[3 * P:6 * P, :]))
                nc.scalar.dma_start(wq_t[:, 6:8, :], d_major(wqT[6 * P:, :]))
                xT_t = xpool.tile([P, ND, S], BF16, tag="xT", name="xT")
                nc.scalar.dma_start(xT_t[:, :, 0:512], d_major(xT[:, 0:512]))
                nc.gpsimd.dma_start(
                    xT_t[:, :, 512:1024], d_major(xT[:, 512:1024])
                )

                # PE warm-up to lift the clock gate while loads land.
                wps = psA.tile([P, 512], F32, tag="wps", name="wps", bufs=1)
                for w in range(64):
                    _mm(nc, wps[:], warm[:, 0:P], warm[:], start=True, stop=True)

                xTj_t = xpool.tile([P, ND, JH], BF16, tag="xTj", name="xTj")
                wk_t = wpool.tile([P, ND, D], BF16, tag="wk", name="wk")
                wv_t = wpool.tile([P, ND, D], BF16, tag="wv", name="wv")

                # --- Q projection: qT[o, :, i] ---
                for ic in range(S // 512):
                    for o in range(NO):
                        pq = psA.tile([P, 512], F32, tag="pp", name=f"pq{ic}_{o}")
                        for d in range(ND):
                            _mm(nc, pq[:],
                                wq_t[:, d, o * P:(o + 1) * P],
                                xT_t[:, d, ic * 512:(ic + 1) * 512],
                                start=(d == 0), stop=(d == ND - 1))
                        if o % 2 == 0:
                            nc.vector.tensor_copy(
                                qT[:, o, ic * 512:(ic + 1) * 512], pq[:])
                        else:
                            nc.scalar.copy(
                                qT[:, o, ic * 512:(ic + 1) * 512], pq[:])
                    # progressive bulk loads, issued behind the compute
                    if ic == 0:
                        nc.sync.dma_start(
                            xT_t[:, :, 1024:1536], d_major(xT[:, 1024:1536])
                        )
                    elif ic == 1:
                        nc.gpsimd.dma_start(
                            xT_t[:, :, 1536:2048], d_major(xT[:, 1536:2048])
                        )
                    elif ic == 2:
                        nc.gpsimd.dma_start(xTj_t[:], d_major(xTj))
                        nc.scalar.dma_start(wk_t[:], d_major(wkT))

                # --- K projection: kT[o, :, j_local] ---
                for jc in range(JH // 512):
                    for o in range(NO):
                        pk = psA.tile([P, 512], F32, tag="pp", name=f"pk{jc}_{o}")
                        for d in range(ND):
                            _mm(nc, pk[:],
                                wk_t[:, d, o * P:(o + 1) * P],
                                xTj_t[:, d, jc * 512:(jc + 1) * 512],
                                start=(d == 0), stop=(d == ND - 1))
                        if o % 2 == 0:
                            nc.vector.tensor_copy(
                                kT[:, o, jc * 512:(jc + 1) * 512], pk[:])
                        else:
                            nc.scalar.copy(
                                kT[:, o, jc * 512:(jc + 1) * 512], pk[:])
                    if jc == 0:
                        nc.gpsimd.dma_start(wv_t[:], d_major(wvT))

                # --- V projection: v[j 128, t, o] ---
                for t in range(NJT):
                    for ob in range(2):
                        pv = psA.tile([P, 512], F32, tag="pp", name=f"pv{t}_{ob}")
                        for d in range(ND):
                            _mm(nc, pv[:],
                                xTj_t[:, d, t * P:(t + 1) * P],
                                wv_t[:, d, ob * 512:(ob + 1) * 512],
                                start=(d == 0), stop=(d == ND - 1))
                        if ob % 2 == 0:
                            nc.vector.tensor_copy(
                                v_t[:, t, ob * 512:(ob + 1) * 512], pv[:])
                        else:
                            nc.scalar.copy(
                                v_t[:, t, ob * 512:(ob + 1) * 512], pv[:])

            # ---------------- Phase B: attention ----------------
            with (
                tc.tile_pool(name="ex", bufs=4) as expool,
                tc.tile_pool(name="ost", bufs=4) as ostpool,
                tc.tile_pool(name="dst", bufs=4) as dstpool,
                tc.tile_pool(name="psS", bufs=2, space="PSUM") as psS,
                tc.tile_pool(name="psC", bufs=1, space="PSUM") as psC,
                tc.tile_pool(name="psD", bufs=1, space="PSUM") as psD,
            ):
                # largest block first => the last block processed is tiny,
                # so little output DMA is exposed at the end of the kernel
                for ib in reversed(range(N_IB)):
                    njt = ib + 1
                    cps = [
                        [
                            psC.tile([P, 512], F32, tag=f"c{it}{ob}",
                                     name=f"c{ib}_{it}{ob}")
                            for ob in range(2)
                        ]
                        for it in range(2)
                    ]
                    dps = [
                        psD.tile([P, 2], F32, tag=f"d{it}", name=f"d{ib}_{it}")
                        for it in range(2)
                    ]

                    def issue_av(t, et):
                        last = t == njt - 1
                        for it in range(2):
                            lhs = et[:, it * P:(it + 1) * P]
                            for ob in range(2):
                                _mm(nc, cps[it][ob][:], lhs,
                                    v_t[:, t, ob * 512:(ob + 1) * 512],
                                    start=(t == 0), stop=last)
                            _mm(nc, dps[it][:], lhs, ones_t[:],
                                start=(t == 0), stop=last)

                    # software pipeline: AV(t-1) issued after scores(t),
                    # giving exp(t-1) the scores(t) window to complete
                    et_prev = None
                    for t in range(njt):
                        ps = psS.tile([P, IB], F32, tag="ps", name=f"ps{ib}_{t}")
                        for o in range(NO):
                            _mm(nc, ps[:],
                                kT[:, o, t * P:(t + 1) * P],
                                qT[:, o, ib * IB:(ib + 1) * IB],
                                start=(o == 0), stop=(o == NO - 1))
                        et = expool.tile([P, IB], BF16, tag="et",
                                         name=f"et{ib}_{t}")
                        nc.scalar.activation(
                            et[:], ps[:],
                            mybir.ActivationFunctionType.Exp, scale=scale,
                        )
                        if t == njt - 1:
                            etm = expool.tile([P, IB], BF16, tag="md",
                                              name=f"md{ib}")
                            nc.vector.tensor_mul(etm[:], et[:], mask_t[:])
                            et = etm
                        if et_prev is not None:
                            issue_av(t - 1, et_prev)
                        et_prev = et
                    issue_av(njt - 1, et_prev)

                    for it in range(2):
                        row0 = ib * IB + it * P
                        ds = dstpool.tile([P, 2], F32, tag="ds",
                                          name=f"ds{ib}_{it}")
                        nc.vector.tensor_copy(ds[:], dps[it][:])
                        nc.gpsimd.dma_start(den_out[row0:row0 + P, :], ds[:])
                        ot = ostpool.tile([P, D], BF16, tag="ot",
                                          name=f"ot{ib}_{it}")
                        eng = nc.sync if it == 0 else nc.gpsimd
                        nc.vector.tensor_copy(ot[:, 0:512], cps[it][0][:])
                        eng.dma_start(ctx_out[row0:row0 + P, 0:512],
                                      ot[:, 0:512])
                        nc.vector.tensor_copy(ot[:, 512:1024], cps[it][1][:])
                        eng.dma_start(ctx_out[row0:row0 + P, 512:1024],
                                      ot[:, 512:1024])

    nc.compile()
    return nc


_NC_CACHE = None


def _get_nc():
    global _NC_CACHE
    if _NC_CACHE is None:
        _NC_CACHE = build_program()
    return _NC_CACHE


def make_core_inputs(x, Wq, Wk, Wv):
    """Host-side shard prep. Returns list of 8 in_maps."""
    x = np.asarray(x, dtype=np.float32)
    wqT = np.ascontiguousarray(np.asarray(Wq, np.float32).T).astype(NPBF16)
    wkT = np.ascontiguousarray(np.asarray(Wk, np.float32).T).astype(NPBF16)
    wvT = np.ascontiguousarray(np.asarray(Wv, np.float32).T).astype(NPBF16)
    ones = np.ones((P, 2), NPBF16)

    # mask[jj, ii] for the diagonal j-tile: keep jj - ii <= delta
    jj = np.arange(P, dtype=np.float32)[:, None]
    ii = np.arange(IB, dtype=np.float32)[None, :]
    masks = [
        ((jj - ii) <= delta).astype(NPBF16) for delta in (0.0, -128.0)
    ]

    in_maps = []
    for c in range(N_CORES):
        b, r = divmod(c, 2)
        xb = x[b]                          # [S, D] fp32
        xT = np.ascontiguousarray(xb.T).astype(NPBF16)       # [D, S]
        # own key rows: global j-tiles 2t + r -> rows [ (2t+r)*128, +128 )
        rows = np.concatenate(
            [np.arange((2 * t + r) * P, (2 * t + r + 1) * P) for t in range(NJT)]
        )
        xTj = np.ascontiguousarray(xb[rows, :].T).astype(NPBF16)  # [D, JH]
        in_maps.append({
            "xT": xT, "xTj": xTj,
            "wqT": wqT, "wkT": wkT, "wvT": wvT,
            "mask": masks[r], "ones": ones,
        })
    return in_maps


def assemble_output(results):
    """Combine per-core partial (ctx, den) into the full [B, S, D] output."""
    out = np.empty((B, S, D), np.float32)
    for b in range(B):
        ctx_e = np.asarray(results[2 * b]["ctx"]).astype(np.float32)
        ctx_o = np.asarray(results[2 * b + 1]["ctx"]).astype(np.float32)
        den_e = np.asarray(results[2 * b]["den"])[:, 0:1]
        den_o = np.asarray(results[2 * b + 1]["den"])[:, 0:1]
        out[b] = (ctx_e + ctx_o) / (den_e + den_o)
    return out


def kernel(x, Wq, Wk, Wv):
    nc = _get_nc()
    in_maps = make_core_inputs(x, Wq, Wk, Wv)
    res = run_bass_kernel_spmd(nc, in_maps, list(range(N_CORES)))
    return assemble_output(res.results)


# revision 12
# speedup vs baseline: 1.6257x; 1.0298x over previous
"""
Causal self-attention (single head) on 8 trn2 NeuronCores.

Problem: x[4, 2048, 1024], Wq/Wk/Wv[1024, 1024] (torch Linear layout
[d_out, d_in]).
    q/k/v = x @ W.T ; out = softmax(mask(q k^T) / 32) @ v

Sharding — flash-style key split (no collectives, uniform SPMD
program; all role differences live in the INPUTS):
  core c -> batch b = c // 2, role r = c % 2.
  Keys/values are split between the pair by alternating 128-row
  j-tiles: core r owns global j-tiles {2t + r}.  Each core projects
  K/V only for its own 1024 key rows, Q for all 2048 query rows (the
  only duplicated work).  Each core computes partial ctx/den over ITS
  keys for ALL queries; the host combines
      out = (ctxE + ctxO) / (denE + denO).
  exp needs no running-max (logits/32 are O(2)).

  Host-side column permutation: x columns (sequence) are reordered
  OWN-tiles-first [own 8 x 128 | other 8 x 128].  Then K/V projection
  reads xT[:, 0:1024] on every core (uniform program), and query
  block ib consists of permuted row-tiles {ib, ib+8} on both roles,
  selected with one strided access pattern.  ctx/den are produced in
  permuted row order; the host un-permutes.  The causal mask for the
  diagonal j-tile is constant per role:
    r=0: [tril | keep-all]   r=1: [tril | drop-all]  (bf16 0/1 input)

All matmuls are bf16 (~0.43 ns/row at N=512, LDWEIGHTS hidden); PSUM
accumulates fp32.  End-to-end rel err ~5e-3 vs the 2e-2 gate (fp8
measured 1.8-2.1e-2 — too close).

Schedule notes (from perfetto traces):
 - DMA engines only start moving ~10 us into the kernel; a tiny warm
   tile rides FIRST on the sync queue and feeds ~64 PE warm-up
   matmuls that both cover the DMA spin-up and lift the clock gate.
 - Attention is software-pipelined: AV(prev) is issued AFTER
   scores(cur), so exp(prev) (scalar engine) completes under the
   scores window.  In-block tile order is [0, diag, 1, 2, ...]: the
   diagonal tile's exp -> mask-mult chain gets two windows of cover,
   and the first AV (which waits on the PREVIOUS block's ctx-copy
   freeing the PSUM bank) gets the same.
 - i-blocks run largest-first so the final block's output DMA is tiny.
 - den partials accumulate in one resident SBUF tile, DMA'd once.
"""

import sys

for _p in ("/opt/trn_rl_repo", "/root/.axon_site/_ro/trn_rl_repo"):
    if _p not in sys.path:
        sys.path.append(_p)

import numpy as np
import ml_dtypes

import concourse.bass as bass
import concourse.mybir as mybir
import concourse.tile as tile
from concourse import bacc
from concourse.bass_utils import run_bass_kernel_spmd

F32 = mybir.dt.float32
BF16 = mybir.dt.bfloat16
NPBF16 = ml_dtypes.bfloat16

B, S, D = 4, 2048, 1024
P = 128
ND = D // P          # 8 d-tiles (projection contraction)
NO = D // P          # 8 o-tiles
IB = 256             # query block rows
N_IB = S // IB       # 8 query blocks
JH = S // 2          # 1024 own key rows per core
NJT = JH // P        # 8 own j-tiles
N_CORES = 8


def _mm(nc, out, lhsT, rhs, start, stop):
    nc.tensor.matmul(out, lhsT, rhs, start=start, stop=stop)


def build_program():
    nc = bacc.Bacc(
        "TRN2",
        target_bir_lowering=False,
        debug=False,
        enable_asserts=False,
        num_devices=N_CORES,
    )
    xT = nc.dram_tensor("xT", [D, S], BF16, kind="ExternalInput").ap()
    wqT = nc.dram_tensor("wqT", [D, D], BF16, kind="ExternalInput").ap()
    wkT = nc.dram_tensor("wkT", [D, D], BF16, kind="ExternalInput").ap()
    wvT = nc.dram_tensor("wvT", [D, D], BF16, kind="ExternalInput").ap()
    mask_in = nc.dram_tensor("mask", [P, IB], BF16, kind="ExternalInput").ap()
    ones_in = nc.dram_tensor("ones", [P, 2], BF16, kind="ExternalInput").ap()
    ctx_out = nc.dram_tensor("ctx", [S, D], BF16, kind="ExternalOutput").ap()
    den_out = nc.dram_tensor("den", [P, 32], F32, kind="ExternalOutput").ap()

    scale = 1.0 / 32.0  # 1/sqrt(d_v)

    def d_major(ap2d):
        # [ND*P, C] DRAM view -> [P, ND, C] (partition-major 3D AP)
        return ap2d.rearrange("(nd p) c -> p nd c", p=P)

    with tile.TileContext(nc) as tc:
        with (
            tc.tile_pool(name="const", bufs=1) as cpool,
            tc.tile_pool(name="res", bufs=1) as rpool,
        ):
            # warm tile rides first on the sync queue so the PE can start
            # ramping as soon as the DMA engines spin up (~10 us).
            warm = cpool.tile([P, 512], BF16, tag="warm", name="warm")
            nc.sync.dma_start(warm[:], xT[0:P, 0:512])
            mask_t = cpool.tile([P, IB], BF16, tag="mask")
            nc.sync.dma_start(mask_t[:], mask_in[:])
            ones_t = cpool.tile([P, 2], BF16, tag="ones")
            nc.sync.dma_start(ones_t[:], ones_in[:])

            qT = rpool.tile([P, NO, S], BF16, tag="qT", name="qT")
            kT = rpool.tile([P, NO, JH], BF16, tag="kT", name="kT")
            v_t = rpool.tile([P, NJT, D], BF16, tag="v", name="v")
            den_all = rpool.tile([P, 32], F32, tag="den", name="den_all")

            # ---------------- Phase A: projections ----------------
            with (
                tc.tile_pool(name="xp", bufs=1) as xpool,
                tc.tile_pool(name="wp", bufs=1) as wpool,
                tc.tile_pool(name="psA", bufs=3, space="PSUM") as psA,
            ):
                # Priority loads: wq striped over the 3 DMA queues, then
                # the xT chunks; wk/wv behind them on the queues.
                wq_t = wpool.tile([P, ND, D], BF16, tag="wq", name="wq")
                nc.sync.dma_start(wq_t[:, 0:3, :], d_major(wqT[0:3 * P, :]))
                nc.gpsimd.dma_start(wq_t[:, 3:6, :], d_major(wqT[3 * P:6 * P, :]))
                nc.scalar.dma_start(wq_t[:, 6:8, :], d_major(wqT[6 * P:, :]))
                xT_t = xpool.tile([P, ND, S], BF16, tag="xT", name="xT")
                nc.scalar.dma_start(xT_t[:, :, 0:512], d_major(xT[:, 0:512]))
                nc.gpsimd.dma_start(
                    xT_t[:, :, 512:1024], d_major(xT[:, 512:1024])
                )
                nc.sync.dma_start(
                    xT_t[:, :, 1024:1536], d_major(xT[:, 1024:1536])
                )

                # PE warm-up (covers DMA spin-up + clock ramp)
                wps = psA.tile([P, 512], F32, tag="wps", name="wps", bufs=1)
                for w in range(64):
                    _mm(nc, wps[:], warm[:, 0:P], warm[:], start=True, stop=True)

                nc.gpsimd.dma_start(
                    xT_t[:, :, 1536:2048], d_major(xT[:, 1536:2048])
                )
                wk_t = wpool.tile([P, ND, D], BF16, tag="wk", name="wk")
                nc.scalar.dma_start(wk_t[:], d_major(wkT))
                wv_t = wpool.tile([P, ND, D], BF16, tag="wv", name="wv")
                nc.gpsimd.dma_start(wv_t[:], d_major(wvT))

                # --- Q projection: qT[o, :, i_perm] ---
                for ic in range(S // 512):
                    for o in range(NO):
                        pq = psA.tile([P, 512], F32, tag="pp", name=f"pq{ic}_{o}")
                        for d in range(ND):
                            _mm(nc, pq[:],
                                wq_t[:, d, o * P:(o + 1) * P],
                                xT_t[:, d, ic * 512:(ic + 1) * 512],
                                start=(d == 0), stop=(d == ND - 1))
                        if o % 2 == 0:
                            nc.vector.tensor_copy(
                                qT[:, o, ic * 512:(ic + 1) * 512], pq[:])
                        else:
                            nc.scalar.copy(
                                qT[:, o, ic * 512:(ic + 1) * 512], pq[:])

                # --- K projection over own keys: kT[o, :, j_local] ---
                for jc in range(JH // 512):
                    for o in range(NO):
                        pk = psA.tile([P, 512], F32, tag="pp", name=f"pk{jc}_{o}")
                        for d in range(ND):
                            _mm(nc, pk[:],
                                wk_t[:, d, o * P:(o + 1) * P],
                                xT_t[:, d, jc * 512:(jc + 1) * 512],
                                start=(d == 0), stop=(d == ND - 1))
                        if o % 2 == 0:
                            nc.vector.tensor_copy(
                                kT[:, o, jc * 512:(jc + 1) * 512], pk[:])
                        else:
                            nc.scalar.copy(
                                kT[:, o, jc * 512:(jc + 1) * 512], pk[:])

                # --- V projection over own keys: v[j 128, t, o] ---
                for t in range(NJT):
                    for ob in range(2):
                        pv = psA.tile([P, 512], F32, tag="pp", name=f"pv{t}_{ob}")
                        for d in range(ND):
                            _mm(nc, pv[:],
                                xT_t[:, d, t * P:(t + 1) * P],
                                wv_t[:, d, ob * 512:(ob + 1) * 512],
                                start=(d == 0), stop=(d == ND - 1))
                        if ob % 2 == 0:
                            nc.vector.tensor_copy(
                                v_t[:, t, ob * 512:(ob + 1) * 512], pv[:])
                        else:
                            nc.scalar.copy(
                                v_t[:, t, ob * 512:(ob + 1) * 512], pv[:])

            # ---------------- Phase B: attention ----------------
            with (
                tc.tile_pool(name="ex", bufs=4) as expool,
                tc.tile_pool(name="ost", bufs=4) as ostpool,
                tc.tile_pool(name="psS", bufs=2, space="PSUM") as psS,
                tc.tile_pool(name="psC", bufs=1, space="PSUM") as psC,
                tc.tile_pool(name="psD", bufs=1, space="PSUM") as psD,
            ):
                # largest block first => the final block's exposed output
                # DMA is minimal
                for ib in reversed(range(N_IB)):
                    njt = ib + 1
                    # tile processing order: [0, diag, 1, 2, ...] gives the
                    # diagonal exp->mask chain and the previous block's
                    # PSUM-copy waits two scores windows of cover
                    if njt >= 3:
                        proc = [0, njt - 1] + list(range(1, njt - 1))
                    else:
                        proc = list(range(njt))
                    cps = [
                        [
                            psC.tile([P, 512], F32, tag=f"c{it}{ob}",
                                     name=f"c{ib}_{it}{ob}")
                            for ob in range(2)
                        ]
                        for it in range(2)
                    ]
                    dps = [
                        psD.tile([P, 2], F32, tag=f"d{it}", name=f"d{ib}_{it}")
                        for it in range(2)
                    ]

                    def issue_av(t, et, first, last):
                        for it in range(2):
                            lhs = et[:, it * P:(it + 1) * P]
                            for ob in range(2):
                                _mm(nc, cps[it][ob][:], lhs,
                                    v_t[:, t, ob * 512:(ob + 1) * 512],
                                    start=first, stop=last)
                            _mm(nc, dps[it][:], lhs, ones_t[:],
                                start=first, stop=last)

                    prev = None  # (t, et, first)
                    for idx, t in enumerate(proc):
                        ps = psS.tile([P, IB], F32, tag="ps", name=f"ps{ib}_{t}")
                        for o in range(NO):
                            qv = qT[:, o, :].rearrange(
                                "p (h u c) -> p h u c", h=2, c=P
                            )
                            _mm(nc, ps[:],
                                kT[:, o, t * P:(t + 1) * P],
                                qv[:, :, ib, :],
                                start=(o == 0), stop=(o == NO - 1))
                        et = expool.tile([P, IB], BF16, tag="et",
                                         name=f"et{ib}_{t}")
                        nc.scalar.activation(
                            et[:], ps[:],
                            mybir.ActivationFunctionType.Exp, scale=scale,
                        )
                        if t == njt - 1:
                            etm = expool.tile([P, IB], BF16, tag="md",
                                              name=f"md{ib}")
                            nc.vector.tensor_mul(etm[:], et[:], mask_t[:])
                            et = etm
                        if prev is not None:
                            issue_av(prev[0], prev[1], prev[2], False)
                        prev = (t, et, idx == 0)
                    issue_av(prev[0], prev[1], prev[2], True)

                    for it in range(2):
                        p_tile = it * 8 + ib          # permuted row-tile
                        row0 = p_tile * P
                        col = 2 * (2 * ib + it)
                        nc.vector.tensor_copy(
                            den_all[:, col:col + 2], dps[it][:])
                        ot = ostpool.tile([P, D], BF16, tag="ot",
                                          name=f"ot{ib}_{it}")
                        eng = nc.sync if it == 0 else nc.gpsimd
                        nc.vector.tensor_copy(ot[:, 0:512], cps[it][0][:])
                        eng.dma_start(ctx_out[row0:row0 + P, 0:512],
                                      ot[:, 0:512])
                        nc.vector.tensor_copy(ot[:, 512:1024], cps[it][1][:])
                        eng.dma_start(ctx_out[row0:row0 + P, 512:1024],
                                      ot[:, 512:1024])
                nc.scalar.dma_start(den_out[:], den_all[:])

    nc.compile()
    return nc


_NC_CACHE = None


def _get_nc():
    global _NC_CACHE
    if _NC_CACHE is None:
        _NC_CACHE = build_program()
    return _NC_CACHE


def _perm_rows(r):
    """Permuted row order: own j-tiles first, then the partner's."""
    own = [2 * t + r for t in range(NJT)]
    other = [2 * t + (1 - r) for t in range(NJT)]
    tiles = own + other
    return np.concatenate(
        [np.arange(g * P, (g + 1) * P) for g in tiles]
    )


def make_core_inputs(x, Wq, Wk, Wv):
    """Host-side shard prep. Returns list of 8 in_maps."""
    x = np.asarray(x, dtype=np.float32)
    wqT = np.ascontiguousarray(np.asarray(Wq, np.float32).T).astype(NPBF16)
    wkT = np.ascontiguousarray(np.asarray(Wk, np.float32).T).astype(NPBF16)
    wvT = np.ascontiguousarray(np.asarray(Wv, np.float32).T).astype(NPBF16)
    ones = np.ones((P, 2), NPBF16)

    # diagonal-tile masks [jj, ii] over i = [self-tile | partner-tile]:
    #   r=0: [tril | keep-all]   r=1: [tril | drop-all]
    jj = np.arange(P, dtype=np.float32)[:, None]
    ii = np.arange(P, dtype=np.float32)[None, :]
    tril = (jj <= ii).astype(NPBF16)
    masks = [
        np.concatenate([tril, np.ones((P, P), NPBF16)], axis=1),
        np.concatenate([tril, np.zeros((P, P), NPBF16)], axis=1),
    ]

    in_maps = []
    for c in range(N_CORES):
        b, r = divmod(c, 2)
        xp = x[b][_perm_rows(r), :]        # [S, D] fp32, permuted rows
        xT = np.ascontiguousarray(xp.T).astype(NPBF16)       # [D, S]
        in_maps.append({
            "xT": xT,
            "wqT": wqT, "wkT": wkT, "wvT": wvT,
            "mask": masks[r], "ones": ones,
        })
    return in_maps


def assemble_output(results):
    """Combine per-core partial (ctx, den) into the full [B, S, D] output."""
    out = np.empty((B, S, D), np.float32)
    for b in range(B):
        num = np.zeros((S, D), np.float32)
        den = np.zeros((S, 1), np.float32)
        for r in range(2):
            res = results[2 * b + r]
            ctx_p = np.asarray(res["ctx"]).astype(np.float32)   # [S, D] perm
            den_p = np.asarray(res["den"])                       # [P, 32]
            inv = _perm_rows(r)                # permuted pos -> global row
            num[inv] += ctx_p
            # den slot for permuted tile p: p = it*8 + ib, col = 2*(2*ib+it)
            dv = np.empty((S,), np.float32)
            for ib in range(N_IB):
                for it in range(2):
                    p_tile = it * 8 + ib
                    col = 2 * (2 * ib + it)
                    dv[p_tile * P:(p_tile + 1) * P] = den_p[:, col]
            den[inv, 0] += dv
        out[b] = num / den
    return out


def kernel(x, Wq, Wk, Wv):
    nc = _get_nc()
    in_maps = make_core_inputs(x, Wq, Wk, Wv)
    res = run_bass_kernel_spmd(nc, in_maps, list(range(N_CORES)))
    return assemble_output(res.results)


# revision 14
# speedup vs baseline: 1.8684x; 1.1493x over previous
"""
Causal self-attention (single head) on 8 trn2 NeuronCores.

Problem: x[4, 2048, 1024], Wq/Wk/Wv[1024, 1024] (torch Linear layout
[d_out, d_in]).
    q/k/v = x @ W.T ; out = softmax(mask(q k^T) / 32) @ v

Sharding — flash-style key split (no collectives, uniform SPMD
program; all role differences live in the INPUTS):
  core c -> batch b = c // 2, role r = c % 2.
  Keys/values are split between the pair by alternating 128-row
  j-tiles: core r owns global j-tiles {2t + r}.  Each core projects
  K/V only for its own 1024 key rows, Q for all 2048 query rows (the
  only duplicated work).  Each core computes partial ctx/den over ITS
  keys for ALL queries; the host combines
      out = (ctxE + ctxO) / (denE + denO).
  exp needs no running-max (logits/32 are O(2)).

  Host-side column permutation: x columns (sequence) are reordered
  OWN-tiles-first [own 8 x 128 | other 8 x 128].  Then K/V projection
  reads xT[:, 0:1024] on every core (uniform program), and query
  block ib consists of permuted row-tiles {ib, ib+8} on both roles,
  selected with one strided access pattern.  ctx/den are produced in
  permuted row order; the host un-permutes.  The causal mask for the
  diagonal j-tile is constant per role:
    r=0: [tril | keep-all]   r=1: [tril | drop-all]  (bf16 0/1 input)

Precision: the Q projection runs in fp8 e4m3 with the DoubleRow perf
mode (2 contraction tiles per instruction, 2x bf16 throughput; Wq is
pre-scaled by 32 so its sigma=0.64 values clear the e4m3 subnormal
cliff, and the exp scale absorbs the 32).  Everything else is bf16
(~0.43 ns/row at N=512, LDWEIGHTS hidden); PSUM accumulates fp32.
Simulated end-to-end rel err: 1.41e-2 vs the 2e-2 gate (bf16-only is
4.7e-3; fp8 on both q&k or on scores measured 1.8-2.1e-2 — too hot).
The HW bf16 result matched the numpy simulation to 5 digits, so the
sim is trusted and the HW error is re-verified by test.py.

Schedule notes (from perfetto traces):
 - DMA engines only start moving ~10 us into the kernel; the warm-up
   tile is memset on-chip (no DMA), so the PE clock gate lifts during
   the dead window.  Critical loads (wq8 + first x8 chunk) ride first
   on their queues; bulk loads are ordered behind by need date.
 - Attention is software-pipelined: AV(prev) is issued AFTER
   scores(cur) so exp(prev) completes under the scores window; tile
   order [0, diag, 1, ...] gives the diagonal's exp->mask chain and
   the previous block's PSUM-copy wait two windows of cover.
 - i-blocks run largest-first so the final block's output DMA is tiny;
   ctx copies split vector/scalar; den partials accumulate in one
   resident SBUF tile, DMA'd once at the end.
"""

import sys

for _p in ("/opt/trn_rl_repo", "/root/.axon_site/_ro/trn_rl_repo"):
    if _p not in sys.path:
        sys.path.append(_p)

import numpy as np
import ml_dtypes

import concourse.bass as bass
import concourse.mybir as mybir
import concourse.tile as tile
from concourse import bacc
from concourse.bass_utils import run_bass_kernel_spmd

F32 = mybir.dt.float32
BF16 = mybir.dt.bfloat16
FP8 = mybir.dt.float8e4
NPBF16 = ml_dtypes.bfloat16
NPFP8 = ml_dtypes.float8_e4m3
DR = mybir.MatmulPerfMode.DoubleRow

B, S, D = 4, 2048, 1024
P = 128
ND = D // P          # 8 d-tiles (projection contraction)
NO = D // P          # 8 o-tiles
IB = 256             # query block rows
N_IB = S // IB       # 8 query blocks
JH = S // 2          # 1024 own key rows per core
NJT = JH // P        # 8 own j-tiles
N_CORES = 8
WQ_SCALE = 32.0      # pre-scale on Wq so fp8 values clear subnormals


def _mm(nc, out, lhsT, rhs, start, stop):
    nc.tensor.matmul(out, lhsT, rhs, start=start, stop=stop)


def build_program():
    nc = bacc.Bacc(
        "TRN2",
        target_bir_lowering=False,
        debug=False,
        enable_asserts=False,
        num_devices=N_CORES,
    )
    # x8: full permuted sequence, fp8 (Q projection);
    # xT: own key half only, bf16 (K/V projections)
    x8 = nc.dram_tensor("x8", [D, S], FP8, kind="ExternalInput").ap()
    xT = nc.dram_tensor("xT", [D, JH], BF16, kind="ExternalInput").ap()
    wq8 = nc.dram_tensor("wq8", [D, D], FP8, kind="ExternalInput").ap()
    wkT = nc.dram_tensor("wkT", [D, D], BF16, kind="ExternalInput").ap()
    wvT = nc.dram_tensor("wvT", [D, D], BF16, kind="ExternalInput").ap()
    mask_in = nc.dram_tensor("mask", [P, IB], BF16, kind="ExternalInput").ap()
    ones_in = nc.dram_tensor("ones", [P, 2], BF16, kind="ExternalInput").ap()
    ctx_out = nc.dram_tensor("ctx", [S, D], BF16, kind="ExternalOutput").ap()
    den_out = nc.dram_tensor("den", [P, 32], F32, kind="ExternalOutput").ap()

    scale = 1.0 / (32.0 * WQ_SCALE)  # 1/sqrt(d_v) / WQ_SCALE

    def d_major(ap2d):
        # [ND*P, C] DRAM view -> [P, ND, C] (partition-major 3D AP)
        return ap2d.rearrange("(nd p) c -> p nd c", p=P)

    with tile.TileContext(nc) as tc:
        with (
            tc.tile_pool(name="const", bufs=1) as cpool,
            tc.tile_pool(name="res", bufs=1) as rpool,
        ):
            # On-chip warm-up source: no DMA dependency, so the PE clock
            # gate lifts during the DMA spin-up dead window.
            warm = cpool.tile([P, 512], BF16, tag="warm", name="warm")
            nc.vector.memset(warm[:], 0.0)
            mask_t = cpool.tile([P, IB], BF16, tag="mask")
            nc.sync.dma_start(mask_t[:], mask_in[:])
            ones_t = cpool.tile([P, 2], BF16, tag="ones")
            nc.sync.dma_start(ones_t[:], ones_in[:])

            qT = rpool.tile([P, NO, S], BF16, tag="qT", name="qT")
            kT = rpool.tile([P, NO, JH], BF16, tag="kT", name="kT")
            v_t = rpool.tile([P, NJT, D], BF16, tag="v", name="v")
            den_all = rpool.tile([P, 32], F32, tag="den", name="den_all")

            # ---------------- Phase A: projections ----------------
            with (
                tc.tile_pool(name="xp", bufs=1) as xpool,
                tc.tile_pool(name="wp", bufs=1) as wpool,
                tc.tile_pool(name="psA", bufs=3, space="PSUM") as psA,
            ):
                # Queue plan (by need date):
                #  sync:   x8 chunks (Q-proj rhs, needed first)
                #  gpsimd: wq8, xT halves (K/V inputs), wv
                #  scalar: wk
                wq8_t = wpool.tile([P, ND // 2, 2, D], FP8, tag="wq", name="wq")
                nc.gpsimd.dma_start(
                    wq8_t[:],
                    d_major(wq8).rearrange("p (d2 two) c -> p d2 two c", two=2),
                )
                x8_t = xpool.tile([P, ND // 2, 2, S], FP8, tag="x8", name="x8")
                x8_v = d_major(x8).rearrange("p (d2 two) c -> p d2 two c", two=2)
                nc.sync.dma_start(x8_t[:, :, :, 0:512], x8_v[:, :, :, 0:512])
                nc.sync.dma_start(x8_t[:, :, :, 512:2048], x8_v[:, :, :, 512:2048])
                xT_t = xpool.tile([P, ND, JH], BF16, tag="xT", name="xT")
                nc.gpsimd.dma_start(xT_t[:, :, 0:512], d_major(xT[:, 0:512]))
                nc.gpsimd.dma_start(
                    xT_t[:, :, 512:1024], d_major(xT[:, 512:1024])
                )
                wk_t = wpool.tile([P, ND, D], BF16, tag="wk", name="wk")
                nc.scalar.dma_start(wk_t[:], d_major(wkT))
                wv_t = wpool.tile([P, ND, D], BF16, tag="wv", name="wv")
                nc.gpsimd.dma_start(wv_t[:], d_major(wvT))

                # PE warm-up (covers DMA spin-up + clock ramp)
                wps = psA.tile([P, 512], F32, tag="wps", name="wps", bufs=1)
                for w in range(64):
                    _mm(nc, wps[:], warm[:, 0:P], warm[:], start=True, stop=True)

                # --- Q projection (fp8 DoubleRow): qT[o, :, i_perm] ---
                for ic in range(S // 512):
                    for o in range(NO):
                        pq = psA.tile([P, 512], F32, tag="pp", name=f"pq{ic}_{o}")
                        for dp in range(ND // 2):
                            nc.tensor.matmul(
                                pq[:],
                                wq8_t[:, dp, :, o * P:(o + 1) * P],
                                x8_t[:, dp, :, ic * 512:(ic + 1) * 512],
                                start=(dp == 0), stop=(dp == ND // 2 - 1),
                                perf_mode=DR,
                            )
                        if o % 2 == 0:
                            nc.vector.tensor_copy(
                                qT[:, o, ic * 512:(ic + 1) * 512], pq[:])
                        else:
                            nc.scalar.copy(
                                qT[:, o, ic * 512:(ic + 1) * 512], pq[:])

                # --- K projection over own keys: kT[o, :, j_local] ---
                for jc in range(JH // 512):
                    for o in range(NO):
                        pk = psA.tile([P, 512], F32, tag="pp", name=f"pk{jc}_{o}")
                        for d in range(ND):
                            _mm(nc, pk[:],
                                wk_t[:, d, o * P:(o + 1) * P],
                                xT_t[:, d, jc * 512:(jc + 1) * 512],
                                start=(d == 0), stop=(d == ND - 1))
                        if o % 2 == 0:
                            nc.vector.tensor_copy(
                                kT[:, o, jc * 512:(jc + 1) * 512], pk[:])
                        else:
                            nc.scalar.copy(
                                kT[:, o, jc * 512:(jc + 1) * 512], pk[:])

                # --- V projection over own keys: v[j 128, t, o] ---
                for t in range(NJT):
                    for ob in range(2):
                        pv = psA.tile([P, 512], F32, tag="pp", name=f"pv{t}_{ob}")
                        for d in range(ND):
                            _mm(nc, pv[:],
                                xT_t[:, d, t * P:(t + 1) * P],
                                wv_t[:, d, ob * 512:(ob + 1) * 512],
                                start=(d == 0), stop=(d == ND - 1))
                        if ob % 2 == 0:
                            nc.vector.tensor_copy(
                                v_t[:, t, ob * 512:(ob + 1) * 512], pv[:])
                        else:
                            nc.scalar.copy(
                                v_t[:, t, ob * 512:(ob + 1) * 512], pv[:])

            # ---------------- Phase B: attention ----------------
            with (
                tc.tile_pool(name="ex", bufs=4) as expool,
                tc.tile_pool(name="ost", bufs=4) as ostpool,
                tc.tile_pool(name="psS", bufs=2, space="PSUM") as psS,
                tc.tile_pool(name="psC", bufs=1, space="PSUM") as psC,
                tc.tile_pool(name="psD", bufs=1, space="PSUM") as psD,
            ):
                # largest block first => the final block's exposed output
                # DMA is minimal
                for ib in reversed(range(N_IB)):
                    njt = ib + 1
                    # tile order [0, diag, 1, ...]: diagonal exp->mask and
                    # previous-block PSUM-copy waits get two windows
                    if njt >= 3:
                        proc = [0, njt - 1] + list(range(1, njt - 1))
                    else:
                        proc = list(range(njt))
                    cps = [
                        [
                            psC.tile([P, 512], F32, tag=f"c{it}{ob}",
                                     name=f"c{ib}_{it}{ob}")
                            for ob in range(2)
                        ]
                        for it in range(2)
                    ]
                    dps = [
                        psD.tile([P, 2], F32, tag=f"d{it}", name=f"d{ib}_{it}")
                        for it in range(2)
                    ]

                    def issue_av(t, et, first, last):
                        for it in range(2):
                            lhs = et[:, it * P:(it + 1) * P]
                            for ob in range(2):
                                _mm(nc, cps[it][ob][:], lhs,
                                    v_t[:, t, ob * 512:(ob + 1) * 512],
                                    start=first, stop=last)
                            _mm(nc, dps[it][:], lhs, ones_t[:],
                                start=first, stop=last)

                    prev = None  # (t, et, first)
                    for idx, t in enumerate(proc):
                        ps = psS.tile([P, IB], F32, tag="ps", name=f"ps{ib}_{t}")
                        for o in range(NO):
                            qv = qT[:, o, :].rearrange(
                                "p (h u c) -> p h u c", h=2, c=P
                            )
                            _mm(nc, ps[:],
                                kT[:, o, t * P:(t + 1) * P],
                                qv[:, :, ib, :],
                                start=(o == 0), stop=(o == NO - 1))
                        et = expool.tile([P, IB], BF16, tag="et",
                                         name=f"et{ib}_{t}")
                        nc.scalar.activation(
                            et[:], ps[:],
                            mybir.ActivationFunctionType.Exp, scale=scale,
                        )
                        if t == njt - 1:
                            etm = expool.tile([P, IB], BF16, tag="md",
                                              name=f"md{ib}")
                            nc.vector.tensor_mul(etm[:], et[:], mask_t[:])
                            et = etm
                        if prev is not None:
                            issue_av(prev[0], prev[1], prev[2], False)
                        prev = (t, et, idx == 0)
                    issue_av(prev[0], prev[1], prev[2], True)

                    for it in range(2):
                        p_tile = it * 8 + ib          # permuted row-tile
                        row0 = p_tile * P
                        col = 2 * (2 * ib + it)
                        nc.vector.tensor_copy(
                            den_all[:, col:col + 2], dps[it][:])
                        ot = ostpool.tile([P, D], BF16, tag="ot",
                                          name=f"ot{ib}_{it}")
                        eng = nc.sync if it == 0 else nc.gpsimd
                        for ob in range(2):
                            cols = slice(ob * 512, (ob + 1) * 512)
                            if it == 0:
                                nc.vector.tensor_copy(ot[:, cols],
                                                      cps[it][ob][:])
                            else:
                                nc.scalar.copy(ot[:, cols], cps[it][ob][:])
                            eng.dma_start(
                                ctx_out[row0:row0 + P, cols], ot[:, cols])
                nc.scalar.dma_start(den_out[:], den_all[:])

    nc.compile()
    return nc


_NC_CACHE = None


def _get_nc():
    global _NC_CACHE
    if _NC_CACHE is None:
        _NC_CACHE = build_program()
    return _NC_CACHE


def _perm_rows(r):
    """Permuted row order: own j-tiles first, then the partner's."""
    own = [2 * t + r for t in range(NJT)]
    other = [2 * t + (1 - r) for t in range(NJT)]
    tiles = own + other
    return np.concatenate(
        [np.arange(g * P, (g + 1) * P) for g in tiles]
    )


def make_core_inputs(x, Wq, Wk, Wv):
    """Host-side shard prep. Returns list of 8 in_maps."""
    x = np.asarray(x, dtype=np.float32)
    wq8 = np.ascontiguousarray(
        np.asarray(Wq, np.float32).T * WQ_SCALE).astype(NPFP8)
    wkT = np.ascontiguousarray(np.asarray(Wk, np.float32).T).astype(NPBF16)
    wvT = np.ascontiguousarray(np.asarray(Wv, np.float32).T).astype(NPBF16)
    ones = np.ones((P, 2), NPBF16)

    # diagonal-tile masks [jj, ii] over i = [self-tile | partner-tile]:
    #   r=0: [tril | keep-all]   r=1: [tril | drop-all]
    jj = np.arange(P, dtype=np.float32)[:, None]
    ii = np.arange(P, dtype=np.float32)[None, :]
    tril = (jj <= ii).astype(NPBF16)
    masks = [
        np.concatenate([tril, np.ones((P, P), NPBF16)], axis=1),
        np.concatenate([tril, np.zeros((P, P), NPBF16)], axis=1),
    ]

    in_maps = []
    for c in range(N_CORES):
        b, r = divmod(c, 2)
        xp = x[b][_perm_rows(r), :]        # [S, D] fp32, permuted rows
        xpT = xp.T                          # [D, S]
        x8 = np.ascontiguousarray(xpT).astype(NPFP8)
        xT = np.ascontiguousarray(xpT[:, 0:JH]).astype(NPBF16)
        in_maps.append({
            "x8": x8, "xT": xT,
            "wq8": wq8, "wkT": wkT, "wvT": wvT,
            "mask": masks[r], "ones": ones,
        })
    return in_maps


def assemble_output(results):
    """Combine per-core partial (ctx, den) into the full [B, S, D] output."""
    out = np.empty((B, S, D), np.float32)
    for b in range(B):
        num = np.zeros((S, D), np.float32)
        den = np.zeros((S, 1), np.float32)
        for r in range(2):
            res = results[2 * b + r]
            ctx_p = np.asarray(res["ctx"]).astype(np.float32)   # [S, D] perm
            den_p = np.asarray(res["den"])                       # [P, 32]
            inv = _perm_rows(r)                # permuted pos -> global row
            num[inv] += ctx_p
            # den slot for permuted tile p: p = it*8 + ib, col = 2*(2*ib+it)
            dv = np.empty((S,), np.float32)
            for ib in range(N_IB):
                for it in range(2):
                    p_tile = it * 8 + ib
                    col = 2 * (2 * ib + it)
                    dv[p_tile * P:(p_tile + 1) * P] = den_p[:, col]
            den[inv, 0] += dv
        out[b] = num / den
    return out


def kernel(x, Wq, Wk, Wv):
    nc = _get_nc()
    in_maps = make_core_inputs(x, Wq, Wk, Wv)
    res = run_bass_kernel_spmd(nc, in_maps, list(range(N_CORES)))
    return assemble_output(res.results)


# revision 15
# speedup vs baseline: 1.9664x; 1.0525x over previous
"""
Causal self-attention (single head) on 8 trn2 NeuronCores.

Problem: x[4, 2048, 1024], Wq/Wk/Wv[1024, 1024] (torch Linear layout
[d_out, d_in]).
    q/k/v = x @ W.T ; out = softmax(mask(q k^T) / 32) @ v

Sharding — flash-style key split (no collectives, uniform SPMD
program; all role differences live in the INPUTS):
  core c -> batch b = c // 2, role r = c % 2.
  Keys/values are split between the pair by alternating 128-row
  j-tiles: core r owns global j-tiles {2t + r}.  Each core projects
  K/V only for its own 1024 key rows, Q for all 2048 query rows (the
  only duplicated work).  Each core computes partial ctx/den over ITS
  keys for ALL queries; the host combines
      out = (ctxE + ctxO) / (denE + denO).
  exp needs no running-max (logits/32 are O(2)).

  Host-side column permutation: x columns (sequence) are reordered
  OWN-tiles-first [own 8 x 128 | other 8 x 128].  Then K/V projection
  reads xT[:, 0:1024] on every core (uniform program), and query
  block ib consists of permuted row-tiles {ib, ib+8} on both roles,
  selected with one strided access pattern.  ctx/den are produced in
  permuted row order; the host un-permutes.  The causal mask for the
  diagonal j-tile is constant per role:
    r=0: [tril | keep-all]   r=1: [tril | drop-all]  (bf16 0/1 input)

Precision: the Q projection runs in fp8 e4m3 with the DoubleRow perf
mode (2 contraction tiles per instruction, 2x bf16 throughput; Wq is
pre-scaled by 32 so its sigma=0.64 values clear the e4m3 subnormal
cliff, and the exp scale absorbs the 32).  Everything else is bf16
(~0.43 ns/row at N=512, LDWEIGHTS hidden); PSUM accumulates fp32.
Simulated end-to-end rel err: 1.41e-2 vs the 2e-2 gate (bf16-only is
4.7e-3; fp8 on both q&k or on scores measured 1.8-2.1e-2 — too hot).
The HW bf16 result matched the numpy simulation to 5 digits, so the
sim is trusted and the HW error is re-verified by test.py.

Schedule notes (from perfetto traces):
 - DMA engines only start moving ~10 us into the kernel; the warm-up
   tile is memset on-chip (no DMA), so the PE clock gate lifts during
   the dead window.  Critical loads (wq8 + first x8 chunk) ride first
   on their queues; bulk loads are ordered behind by need date.
 - Attention is software-pipelined: AV(prev) is issued AFTER
   scores(cur) so exp(prev) completes under the scores window; tile
   order [0, diag, 1, ...] gives the diagonal's exp->mask chain and
   the previous block's PSUM-copy wait two windows of cover.
 - i-blocks run largest-first so the final block's output DMA is tiny;
   ctx copies split vector/scalar; den partials accumulate in one
   resident SBUF tile, DMA'd once at the end.
"""

import sys

for _p in ("/opt/trn_rl_repo", "/root/.axon_site/_ro/trn_rl_repo"):
    if _p not in sys.path:
        sys.path.append(_p)

import numpy as np
import ml_dtypes

import concourse.bass as bass
import concourse.mybir as mybir
import concourse.tile as tile
from concourse import bacc
from concourse.bass_utils import run_bass_kernel_spmd

F32 = mybir.dt.float32
BF16 = mybir.dt.bfloat16
FP8 = mybir.dt.float8e4
NPBF16 = ml_dtypes.bfloat16
NPFP8 = ml_dtypes.float8_e4m3
DR = mybir.MatmulPerfMode.DoubleRow

B, S, D = 4, 2048, 1024
P = 128
ND = D // P          # 8 d-tiles (projection contraction)
NO = D // P          # 8 o-tiles
IB = 256             # query block rows
N_IB = S // IB       # 8 query blocks
JH = S // 2          # 1024 own key rows per core
NJT = JH // P        # 8 own j-tiles
N_CORES = 8
WQ_SCALE = 32.0      # pre-scale on Wq so fp8 values clear subnormals


def _mm(nc, out, lhsT, rhs, start, stop):
    nc.tensor.matmul(out, lhsT, rhs, start=start, stop=stop)


def build_program():
    nc = bacc.Bacc(
        "TRN2",
        target_bir_lowering=False,
        debug=False,
        enable_asserts=False,
        num_devices=N_CORES,
    )
    # x8: full permuted sequence, fp8 (Q projection);
    # xT: own key half only, bf16 (K/V projections)
    x8 = nc.dram_tensor("x8", [D, S], FP8, kind="ExternalInput").ap()
    xT = nc.dram_tensor("xT", [D, JH], BF16, kind="ExternalInput").ap()
    wq8 = nc.dram_tensor("wq8", [D, D], FP8, kind="ExternalInput").ap()
    wkT = nc.dram_tensor("wkT", [D, D], BF16, kind="ExternalInput").ap()
    wvT = nc.dram_tensor("wvT", [D, D], BF16, kind="ExternalInput").ap()
    mask_in = nc.dram_tensor("mask", [P, IB], BF16, kind="ExternalInput").ap()
    ones_in = nc.dram_tensor("ones", [P, 2], BF16, kind="ExternalInput").ap()
    ctx_out = nc.dram_tensor("ctx", [S, D], BF16, kind="ExternalOutput").ap()
    den_out = nc.dram_tensor("den", [P, 32], F32, kind="ExternalOutput").ap()

    scale = 1.0 / (32.0 * WQ_SCALE)  # 1/sqrt(d_v) / WQ_SCALE

    def d_major(ap2d):
        # [ND*P, C] DRAM view -> [P, ND, C] (partition-major 3D AP)
        return ap2d.rearrange("(nd p) c -> p nd c", p=P)

    with tile.TileContext(nc) as tc:
        with (
            tc.tile_pool(name="const", bufs=1) as cpool,
            tc.tile_pool(name="res", bufs=1) as rpool,
        ):
            # On-chip warm-up source: no DMA dependency, so the PE clock
            # gate lifts during the DMA spin-up dead window.
            warm = cpool.tile([P, 512], BF16, tag="warm", name="warm")
            nc.vector.memset(warm[:], 0.0)
            mask_t = cpool.tile([P, IB], BF16, tag="mask")
            nc.sync.dma_start(mask_t[:], mask_in[:])
            ones_t = cpool.tile([P, 2], BF16, tag="ones")
            nc.sync.dma_start(ones_t[:], ones_in[:])

            qT = rpool.tile([P, NO, S], BF16, tag="qT", name="qT")
            kT = rpool.tile([P, NO, JH], BF16, tag="kT", name="kT")
            v_t = rpool.tile([P, NJT, D], BF16, tag="v", name="v")
            den_all = rpool.tile([P, 32], F32, tag="den", name="den_all")

            # ---------------- Phase A: projections ----------------
            with (
                tc.tile_pool(name="xp", bufs=1) as xpool,
                tc.tile_pool(name="wp", bufs=1) as wpool,
                tc.tile_pool(name="psA", bufs=3, space="PSUM") as psA,
            ):
                # Queue plan.  Measured per-queue DMA rates are wildly
                # uneven: gpsimd (software DGE) ~200 GB/s, scalar ~80,
                # sync ~44.  Critical loads ride gpsimd in need order.
                x8_t = xpool.tile([P, ND // 2, 2, S], FP8, tag="x8", name="x8")
                x8_v = d_major(x8).rearrange("p (d2 two) c -> p d2 two c", two=2)
                nc.gpsimd.dma_start(x8_t[:, :, :, 0:512], x8_v[:, :, :, 0:512])
                wq8_t = wpool.tile([P, ND // 2, 2, D], FP8, tag="wq", name="wq")
                nc.gpsimd.dma_start(
                    wq8_t[:],
                    d_major(wq8).rearrange("p (d2 two) c -> p d2 two c", two=2),
                )
                nc.gpsimd.dma_start(
                    x8_t[:, :, :, 512:2048], x8_v[:, :, :, 512:2048]
                )
                xT_t = xpool.tile([P, ND, JH], BF16, tag="xT", name="xT")
                nc.sync.dma_start(xT_t[:, :, 0:512], d_major(xT[:, 0:512]))
                nc.gpsimd.dma_start(
                    xT_t[:, :, 512:1024], d_major(xT[:, 512:1024])
                )
                wk_t = wpool.tile([P, ND, D], BF16, tag="wk", name="wk")
                nc.scalar.dma_start(wk_t[:], d_major(wkT))
                wv_t = wpool.tile([P, ND, D], BF16, tag="wv", name="wv")
                nc.gpsimd.dma_start(wv_t[:], d_major(wvT))

                # PE warm-up (covers DMA spin-up + clock ramp)
                wps = psA.tile([P, 512], F32, tag="wps", name="wps", bufs=1)
                for w in range(64):
                    _mm(nc, wps[:], warm[:, 0:P], warm[:], start=True, stop=True)

                # --- Q projection (fp8 DoubleRow): qT[o, :, i_perm] ---
                for ic in range(S // 512):
                    for o in range(NO):
                        pq = psA.tile([P, 512], F32, tag="pp", name=f"pq{ic}_{o}")
                        for dp in range(ND // 2):
                            nc.tensor.matmul(
                                pq[:],
                                wq8_t[:, dp, :, o * P:(o + 1) * P],
                                x8_t[:, dp, :, ic * 512:(ic + 1) * 512],
                                start=(dp == 0), stop=(dp == ND // 2 - 1),
                                perf_mode=DR,
                            )
                        if o % 2 == 0:
                            nc.vector.tensor_copy(
                                qT[:, o, ic * 512:(ic + 1) * 512], pq[:])
                        else:
                            nc.scalar.copy(
                                qT[:, o, ic * 512:(ic + 1) * 512], pq[:])

                # --- K projection over own keys: kT[o, :, j_local] ---
                for jc in range(JH // 512):
                    for o in range(NO):
                        pk = psA.tile([P, 512], F32, tag="pp", name=f"pk{jc}_{o}")
                        for d in range(ND):
                            _mm(nc, pk[:],
                                wk_t[:, d, o * P:(o + 1) * P],
                                xT_t[:, d, jc * 512:(jc + 1) * 512],
                                start=(d == 0), stop=(d == ND - 1))
                        if o % 2 == 0:
                            nc.vector.tensor_copy(
                                kT[:, o, jc * 512:(jc + 1) * 512], pk[:])
                        else:
                            nc.scalar.copy(
                                kT[:, o, jc * 512:(jc + 1) * 512], pk[:])

                # --- V projection over own keys: v[j 128, t, o] ---
                for t in range(NJT):
                    for ob in range(2):
                        pv = psA.tile([P, 512], F32, tag="pp", name=f"pv{t}_{ob}")
                        for d in range(ND):
                            _mm(nc, pv[:],
                                xT_t[:, d, t * P:(t + 1) * P],
                                wv_t[:, d, ob * 512:(ob + 1) * 512],
                                start=(d == 0), stop=(d == ND - 1))
                        if ob % 2 == 0:
                            nc.vector.tensor_copy(
                                v_t[:, t, ob * 512:(ob + 1) * 512], pv[:])
                        else:
                            nc.scalar.copy(
                                v_t[:, t, ob * 512:(ob + 1) * 512], pv[:])

            # ---------------- Phase B: attention ----------------
            with (
                tc.tile_pool(name="ex", bufs=4) as expool,
                tc.tile_pool(name="ost", bufs=4) as ostpool,
                tc.tile_pool(name="psS", bufs=2, space="PSUM") as psS,
                tc.tile_pool(name="psC", bufs=1, space="PSUM") as psC,
                tc.tile_pool(name="psD", bufs=1, space="PSUM") as psD,
            ):
                # largest block first => the final block's exposed output
                # DMA is minimal
                for ib in reversed(range(N_IB)):
                    njt = ib + 1
                    # tile order [0, diag, 1, ...]: diagonal exp->mask and
                    # previous-block PSUM-copy waits get two windows
                    if njt >= 3:
                        proc = [0, njt - 1] + list(range(1, njt - 1))
                    else:
                        proc = list(range(njt))
                    cps = [
                        [
                            psC.tile([P, 512], F32, tag=f"c{it}{ob}",
                                     name=f"c{ib}_{it}{ob}")
                            for ob in range(2)
                        ]
                        for it in range(2)
                    ]
                    dps = [
                        psD.tile([P, 2], F32, tag=f"d{it}", name=f"d{ib}_{it}")
                        for it in range(2)
                    ]

                    def issue_av(t, et, first, last):
                        for it in range(2):
                            lhs = et[:, it * P:(it + 1) * P]
                            for ob in range(2):
                                _mm(nc, cps[it][ob][:], lhs,
                                    v_t[:, t, ob * 512:(ob + 1) * 512],
                                    start=first, stop=last)
                            _mm(nc, dps[it][:], lhs, ones_t[:],
                                start=first, stop=last)

                    prev = None  # (t, et, first)
                    for idx, t in enumerate(proc):
                        ps = psS.tile([P, IB], F32, tag="ps", name=f"ps{ib}_{t}")
                        for o in range(NO):
                            qv = qT[:, o, :].rearrange(
                                "p (h u c) -> p h u c", h=2, c=P
                            )
                            _mm(nc, ps[:],
                                kT[:, o, t * P:(t + 1) * P],
                                qv[:, :, ib, :],
                                start=(o == 0), stop=(o == NO - 1))
                        et = expool.tile([P, IB], BF16, tag="et",
                                         name=f"et{ib}_{t}")
                        nc.scalar.activation(
                            et[:], ps[:],
                            mybir.ActivationFunctionType.Exp, scale=scale,
                        )
                        if t == njt - 1:
                            etm = expool.tile([P, IB], BF16, tag="md",
                                              name=f"md{ib}")
                            nc.vector.tensor_mul(etm[:], et[:], mask_t[:])
                            et = etm
                        if prev is not None:
                            issue_av(prev[0], prev[1], prev[2], False)
                        prev = (t, et, idx == 0)
                    issue_av(prev[0], prev[1], prev[2], True)

                    for it in range(2):
                        p_tile = it * 8 + ib          # permuted row-tile
                        row0 = p_tile * P
                        col = 2 * (2 * ib + it)
                        nc.vector.tensor_copy(
                            den_all[:, col:col + 2], dps[it][:])
                        ot = ostpool.tile([P, D], BF16, tag="ot",
                                          name=f"ot{ib}_{it}")
                        eng = nc.sync if it == 0 else nc.gpsimd
                        for ob in range(2):
                            cols = slice(ob * 512, (ob + 1) * 512)
                            if it == 0:
                                nc.vector.tensor_copy(ot[:, cols],
                                                      cps[it][ob][:])
                            else:
                                nc.scalar.copy(ot[:, cols], cps[it][ob][:])
                            eng.dma_start(
                                ctx_out[row0:row0 + P, cols], ot[:, cols])
                nc.scalar.dma_start(den_out[:], den_all[:])

    nc.compile()
    return nc


_NC_CACHE = None


def _get_nc():
    global _NC_CACHE
    if _NC_CACHE is None:
        _NC_CACHE = build_program()
    return _NC_CACHE


def _perm_rows(r):
    """Permuted row order: own j-tiles first, then the partner's."""
    own = [2 * t + r for t in range(NJT)]
    other = [2 * t + (1 - r) for t in range(NJT)]
    tiles = own + other
    return np.concatenate(
        [np.arange(g * P, (g + 1) * P) for g in tiles]
    )


def make_core_inputs(x, Wq, Wk, Wv):
    """Host-side shard prep. Returns list of 8 in_maps."""
    x = np.asarray(x, dtype=np.float32)
    wq8 = np.ascontiguousarray(
        np.asarray(Wq, np.float32).T * WQ_SCALE).astype(NPFP8)
    wkT = np.ascontiguousarray(np.asarray(Wk, np.float32).T).astype(NPBF16)
    wvT = np.ascontiguousarray(np.asarray(Wv, np.float32).T).astype(NPBF16)
    ones = np.ones((P, 2), NPBF16)

    # diagonal-tile masks [jj, ii] over i = [self-tile | partner-tile]:
    #   r=0: [tril | keep-all]   r=1: [tril | drop-all]
    jj = np.arange(P, dtype=np.float32)[:, None]
    ii = np.arange(P, dtype=np.float32)[None, :]
    tril = (jj <= ii).astype(NPBF16)
    masks = [
        np.concatenate([tril, np.ones((P, P), NPBF16)], axis=1),
        np.concatenate([tril, np.zeros((P, P), NPBF16)], axis=1),
    ]

    in_maps = []
    for c in range(N_CORES):
        b, r = divmod(c, 2)
        xp = x[b][_perm_rows(r), :]        # [S, D] fp32, permuted rows
        xpT = xp.T                          # [D, S]
        x8 = np.ascontiguousarray(xpT).astype(NPFP8)
        xT = np.ascontiguousarray(xpT[:, 0:JH]).astype(NPBF16)
        in_maps.append({
            "x8": x8, "xT": xT,
            "wq8": wq8, "wkT": wkT, "wvT": wvT,
            "mask": masks[r], "ones": ones,
        })
    return in_maps


def assemble_output(results):
    """Combine per-core partial (ctx, den) into the full [B, S, D] output."""
    out = np.empty((B, S, D), np.float32)
    for b in range(B):
        num = np.zeros((S, D), np.float32)
        den = np.zeros((S, 1), np.float32)
        for r in range(2):
            res = results[2 * b + r]
            ctx_p = np.asarray(res["ctx"]).astype(np.float32)   # [S, D] perm
            den_p = np.asarray(res["den"])                       # [P, 32]
            inv = _perm_rows(r)                # permuted pos -> global row
            num[inv] += ctx_p
            # den slot for permuted tile p: p = it*8 + ib, col = 2*(2*ib+it)
            dv = np.empty((S,), np.float32)
            for ib in range(N_IB):
                for it in range(2):
                    p_tile = it * 8 + ib
                    col = 2 * (2 * ib + it)
                    dv[p_tile * P:(p_tile + 1) * P] = den_p[:, col]
            den[inv, 0] += dv
        out[b] = num / den
    return out


def kernel(x, Wq, Wk, Wv):
    nc = _get_nc()
    in_maps = make_core_inputs(x, Wq, Wk, Wv)
    res = run_bass_kernel_spmd(nc, in_maps, list(range(N_CORES)))
    return assemble_output(res.results)


# revision 16
# speedup vs baseline: 1.9711x; 1.0024x over previous
"""
Causal self-attention (single head) on 8 trn2 NeuronCores.

Problem: x[4, 2048, 1024], Wq/Wk/Wv[1024, 1024] (torch Linear layout
[d_out, d_in]).
    q/k/v = x @ W.T ; out = softmax(mask(q k^T) / 32) @ v

Sharding — flash-style key split (no collectives, uniform SPMD
program; all role differences live in the INPUTS):
  core c -> batch b = c // 2, role r = c % 2.
  Keys/values are split between the pair by alternating 128-row
  j-tiles: core r owns global j-tiles {2t + r}.  Each core projects
  K/V only for its own 1024 key rows, Q for all 2048 query rows (the
  only duplicated work).  Each core computes partial ctx/den over ITS
  keys for ALL queries; the host combines
      out = (ctxE + ctxO) / (denE + denO).
  exp needs no running-max (logits/32 are O(2)).

  Host-side column permutation: x columns (sequence) are reordered
  OWN-tiles-first [own 8 x 128 | other 8 x 128].  Then K/V projection
  reads xT[:, 0:1024] on every core (uniform program), and query
  block ib consists of permuted row-tiles {ib, ib+8} on both roles,
  selected with one strided access pattern.  ctx/den are produced in
  permuted row order; the host un-permutes.  The causal mask for the
  diagonal j-tile is constant per role:
    r=0: [tril | keep-all]   r=1: [tril | drop-all]  (bf16 0/1 input)

Precision: the Q projection runs in fp8 e4m3 with the DoubleRow perf
mode (2 contraction tiles per instruction, 2x bf16 throughput; Wq is
pre-scaled by 32 so its sigma=0.64 values clear the e4m3 subnormal
cliff, and the exp scale absorbs the 32).  Everything else is bf16
(~0.43 ns/row at N=512, LDWEIGHTS hidden); PSUM accumulates fp32.
Simulated end-to-end rel err: 1.41e-2 vs the 2e-2 gate (bf16-only is
4.7e-3; fp8 on both q&k or on scores measured 1.8-2.1e-2 — too hot).
The HW bf16 result matched the numpy simulation to 5 digits, so the
sim is trusted and the HW error is re-verified by test.py.

Schedule notes (from perfetto traces):
 - DMA engines only start moving ~10 us into the kernel; the warm-up
   tile is memset on-chip (no DMA), so the PE clock gate lifts during
   the dead window.  Critical loads (wq8 + first x8 chunk) ride first
   on their queues; bulk loads are ordered behind by need date.
 - Attention is software-pipelined: AV(prev) is issued AFTER
   scores(cur) so exp(prev) completes under the scores window; tile
   order [0, diag, 1, ...] gives the diagonal's exp->mask chain and
   the previous block's PSUM-copy wait two windows of cover.
 - i-blocks run largest-first so the final block's output DMA is tiny;
   ctx copies split vector/scalar; den partials accumulate in one
   resident SBUF tile, DMA'd once at the end.
"""

import sys

for _p in ("/opt/trn_rl_repo", "/root/.axon_site/_ro/trn_rl_repo"):
    if _p not in sys.path:
        sys.path.append(_p)

import numpy as np
import ml_dtypes

import concourse.bass as bass
import concourse.mybir as mybir
import concourse.tile as tile
from concourse import bacc
from concourse.bass_utils import run_bass_kernel_spmd

F32 = mybir.dt.float32
BF16 = mybir.dt.bfloat16
FP8 = mybir.dt.float8e4
NPBF16 = ml_dtypes.bfloat16
NPFP8 = ml_dtypes.float8_e4m3
DR = mybir.MatmulPerfMode.DoubleRow

B, S, D = 4, 2048, 1024
P = 128
ND = D // P          # 8 d-tiles (projection contraction)
NO = D // P          # 8 o-tiles
IB = 256             # query block rows
N_IB = S // IB       # 8 query blocks
JH = S // 2          # 1024 own key rows per core
NJT = JH // P        # 8 own j-tiles
N_CORES = 8
WQ_SCALE = 32.0      # pre-scale on Wq so fp8 values clear subnormals


def _mm(nc, out, lhsT, rhs, start, stop):
    nc.tensor.matmul(out, lhsT, rhs, start=start, stop=stop)


def build_program():
    nc = bacc.Bacc(
        "TRN2",
        target_bir_lowering=False,
        debug=False,
        enable_asserts=False,
        num_devices=N_CORES,
    )
    # x8: full permuted sequence, fp8 (Q projection);
    # xT: own key half only, bf16 (K/V projections)
    x8 = nc.dram_tensor("x8", [D, S], FP8, kind="ExternalInput").ap()
    xT = nc.dram_tensor("xT", [D, JH], BF16, kind="ExternalInput").ap()
    wq8 = nc.dram_tensor("wq8", [D, D], FP8, kind="ExternalInput").ap()
    wkT = nc.dram_tensor("wkT", [D, D], BF16, kind="ExternalInput").ap()
    wvT = nc.dram_tensor("wvT", [D, D], BF16, kind="ExternalInput").ap()
    mask_in = nc.dram_tensor("mask", [P, IB], BF16, kind="ExternalInput").ap()
    ones_in = nc.dram_tensor("ones", [P, 2], BF16, kind="ExternalInput").ap()
    ctx_out = nc.dram_tensor("ctx", [S, D], BF16, kind="ExternalOutput").ap()
    den_out = nc.dram_tensor("den", [P, 32], F32, kind="ExternalOutput").ap()

    scale = 1.0 / (32.0 * WQ_SCALE)  # 1/sqrt(d_v) / WQ_SCALE

    def d_major(ap2d):
        # [ND*P, C] DRAM view -> [P, ND, C] (partition-major 3D AP)
        return ap2d.rearrange("(nd p) c -> p nd c", p=P)

    with tile.TileContext(nc) as tc:
        with tc.tile_pool(name="res", bufs=1) as rpool:
            cpool = rpool
            # On-chip warm-up source: no DMA dependency, so the PE clock
            # gate lifts during the DMA spin-up dead window.
            warm = cpool.tile([P, 512], BF16, tag="warm", name="warm")
            nc.vector.memset(warm[:], 0.0)
            mask_t = cpool.tile([P, IB], BF16, tag="mask")
            nc.sync.dma_start(mask_t[:], mask_in[:])
            ones_t = cpool.tile([P, 2], BF16, tag="ones")
            nc.sync.dma_start(ones_t[:], ones_in[:])

            qT = rpool.tile([P, NO, S], BF16, tag="qT", name="qT")
            kT = rpool.tile([P, NO, JH], BF16, tag="kT", name="kT")
            v_t = rpool.tile([P, NJT, D], BF16, tag="v", name="v")
            den_all = rpool.tile([P, 32], F32, tag="den", name="den_all")

            # ---------------- Phase A: projections ----------------
            with (
                tc.tile_pool(name="xp", bufs=1) as xpool,
                tc.tile_pool(name="wp", bufs=1) as wpool,
                tc.tile_pool(name="psA", bufs=3, space="PSUM") as psA,
            ):
                # Queue plan.  Measured per-queue DMA rates are wildly
                # uneven: gpsimd (software DGE) ~200 GB/s, scalar ~80,
                # sync ~44.  Critical loads ride gpsimd in need order.
                x8_t = xpool.tile([P, ND // 2, 2, S], FP8, tag="x8", name="x8")
                x8_v = d_major(x8).rearrange("p (d2 two) c -> p d2 two c", two=2)
                nc.gpsimd.dma_start(x8_t[:, :, :, 0:512], x8_v[:, :, :, 0:512])
                wq8_t = wpool.tile([P, ND // 2, 2, D], FP8, tag="wq", name="wq")
                nc.gpsimd.dma_start(
                    wq8_t[:],
                    d_major(wq8).rearrange("p (d2 two) c -> p d2 two c", two=2),
                )
                nc.gpsimd.dma_start(
                    x8_t[:, :, :, 512:2048], x8_v[:, :, :, 512:2048]
                )
                xT_t = xpool.tile([P, ND, JH], BF16, tag="xT", name="xT")
                nc.sync.dma_start(xT_t[:, :, 0:512], d_major(xT[:, 0:512]))
                nc.gpsimd.dma_start(
                    xT_t[:, :, 512:1024], d_major(xT[:, 512:1024])
                )
                wk_t = wpool.tile([P, ND, D], BF16, tag="wk", name="wk")
                nc.scalar.dma_start(wk_t[:], d_major(wkT))
                wv_t = wpool.tile([P, ND, D], BF16, tag="wv", name="wv")
                nc.gpsimd.dma_start(wv_t[:], d_major(wvT))

                # PE warm-up (covers DMA spin-up + clock ramp)
                wps = psA.tile([P, 512], F32, tag="wps", name="wps", bufs=1)
                for w in range(64):
                    _mm(nc, wps[:], warm[:, 0:P], warm[:], start=True, stop=True)

                # --- Q projection (fp8 DoubleRow): qT[o, :, i_perm] ---
                for ic in range(S // 512):
                    for o in range(NO):
                        pq = psA.tile([P, 512], F32, tag="pp", name=f"pq{ic}_{o}")
                        for dp in range(ND // 2):
                            nc.tensor.matmul(
                                pq[:],
                                wq8_t[:, dp, :, o * P:(o + 1) * P],
                                x8_t[:, dp, :, ic * 512:(ic + 1) * 512],
                                start=(dp == 0), stop=(dp == ND // 2 - 1),
                                perf_mode=DR,
                            )
                        if o % 2 == 0:
                            nc.vector.tensor_copy(
                                qT[:, o, ic * 512:(ic + 1) * 512], pq[:])
                        else:
                            nc.scalar.copy(
                                qT[:, o, ic * 512:(ic + 1) * 512], pq[:])

                # --- K projection over own keys: kT[o, :, j_local] ---
                for jc in range(JH // 512):
                    for o in range(NO):
                        pk = psA.tile([P, 512], F32, tag="pp", name=f"pk{jc}_{o}")
                        for d in range(ND):
                            _mm(nc, pk[:],
                                wk_t[:, d, o * P:(o + 1) * P],
                                xT_t[:, d, jc * 512:(jc + 1) * 512],
                                start=(d == 0), stop=(d == ND - 1))
                        if o % 2 == 0:
                            nc.vector.tensor_copy(
                                kT[:, o, jc * 512:(jc + 1) * 512], pk[:])
                        else:
                            nc.scalar.copy(
                                kT[:, o, jc * 512:(jc + 1) * 512], pk[:])

                # --- V projection over own keys: v[j 128, t, o] ---
                for t in range(NJT):
                    for ob in range(2):
                        pv = psA.tile([P, 512], F32, tag="pp", name=f"pv{t}_{ob}")
                        for d in range(ND):
                            _mm(nc, pv[:],
                                xT_t[:, d, t * P:(t + 1) * P],
                                wv_t[:, d, ob * 512:(ob + 1) * 512],
                                start=(d == 0), stop=(d == ND - 1))
                        if ob % 2 == 0:
                            nc.vector.tensor_copy(
                                v_t[:, t, ob * 512:(ob + 1) * 512], pv[:])
                        else:
                            nc.scalar.copy(
                                v_t[:, t, ob * 512:(ob + 1) * 512], pv[:])

            # ---------------- Phase B: attention ----------------
            with (
                tc.tile_pool(name="ex", bufs=4) as expool,
                tc.tile_pool(name="psB", bufs=1, space="PSUM") as psB,
            ):
                ostpool = expool
                psS = psC = psD = psB
                # largest block first => the final block's exposed output
                # DMA is minimal
                for ib in reversed(range(N_IB)):
                    njt = ib + 1
                    # tile order [0, diag, 1, ...]: diagonal exp->mask and
                    # previous-block PSUM-copy waits get two windows
                    if njt >= 3:
                        proc = [0, njt - 1] + list(range(1, njt - 1))
                    else:
                        proc = list(range(njt))
                    cps = [
                        [
                            psC.tile([P, 512], F32, tag=f"c{it}{ob}",
                                     name=f"c{ib}_{it}{ob}")
                            for ob in range(2)
                        ]
                        for it in range(2)
                    ]
                    dps = [
                        psD.tile([P, 2], F32, tag=f"d{it}", name=f"d{ib}_{it}")
                        for it in range(2)
                    ]

                    def issue_av(t, et, first, last):
                        for it in range(2):
                            lhs = et[:, it * P:(it + 1) * P]
                            for ob in range(2):
                                _mm(nc, cps[it][ob][:], lhs,
                                    v_t[:, t, ob * 512:(ob + 1) * 512],
                                    start=first, stop=last)
                            _mm(nc, dps[it][:], lhs, ones_t[:],
                                start=first, stop=last)

                    prev = None  # (t, et, first)
                    for idx, t in enumerate(proc):
                        ps = psS.tile([P, IB], F32, tag="ps", name=f"ps{ib}_{t}", bufs=2)
                        for o in range(NO):
                            qv = qT[:, o, :].rearrange(
                                "p (h u c) -> p h u c", h=2, c=P
                            )
                            _mm(nc, ps[:],
                                kT[:, o, t * P:(t + 1) * P],
                                qv[:, :, ib, :],
                                start=(o == 0), stop=(o == NO - 1))
                        et = expool.tile([P, IB], BF16, tag="et",
                                         name=f"et{ib}_{t}")
                        nc.scalar.activation(
                            et[:], ps[:],
                            mybir.ActivationFunctionType.Exp, scale=scale,
                        )
                        if t == njt - 1:
                            etm = expool.tile([P, IB], BF16, tag="md",
                                              name=f"md{ib}")
                            nc.vector.tensor_mul(etm[:], et[:], mask_t[:])
                            et = etm
                        if prev is not None:
                            issue_av(prev[0], prev[1], prev[2], False)
                        prev = (t, et, idx == 0)
                    issue_av(prev[0], prev[1], prev[2], True)

                    for it in range(2):
                        p_tile = it * 8 + ib          # permuted row-tile
                        row0 = p_tile * P
                        col = 2 * (2 * ib + it)
                        nc.vector.tensor_copy(
                            den_all[:, col:col + 2], dps[it][:])
                        ot = ostpool.tile([P, D], BF16, tag="ot",
                                          name=f"ot{ib}_{it}")
                        eng = nc.sync if it == 0 else nc.gpsimd
                        for ob in range(2):
                            cols = slice(ob * 512, (ob + 1) * 512)
                            if it == 0:
                                nc.vector.tensor_copy(ot[:, cols],
                                                      cps[it][ob][:])
                            else:
                                nc.scalar.copy(ot[:, cols], cps[it][ob][:])
                            eng.dma_start(
                                ctx_out[row0:row0 + P, cols], ot[:, cols])
                nc.scalar.dma_start(den_out[:], den_all[:])

    nc.compile()
    return nc


_NC_CACHE = None


def _get_nc():
    global _NC_CACHE
    if _NC_CACHE is None:
        _NC_CACHE = build_program()
    return _NC_CACHE


def _perm_rows(r):
    """Permuted row order: own j-tiles first, then the partner's."""
    own = [2 * t + r for t in range(NJT)]
    other = [2 * t + (1 - r) for t in range(NJT)]
    tiles = own + other
    return np.concatenate(
        [np.arange(g * P, (g + 1) * P) for g in tiles]
    )


def make_core_inputs(x, Wq, Wk, Wv):
    """Host-side shard prep. Returns list of 8 in_maps."""
    x = np.asarray(x, dtype=np.float32)
    wq8 = np.ascontiguousarray(
        np.asarray(Wq, np.float32).T * WQ_SCALE).astype(NPFP8)
    wkT = np.ascontiguousarray(np.asarray(Wk, np.float32).T).astype(NPBF16)
    wvT = np.ascontiguousarray(np.asarray(Wv, np.float32).T).astype(NPBF16)
    ones = np.ones((P, 2), NPBF16)

    # diagonal-tile masks [jj, ii] over i = [self-tile | partner-tile]:
    #   r=0: [tril | keep-all]   r=1: [tril | drop-all]
    jj = np.arange(P, dtype=np.float32)[:, None]
    ii = np.arange(P, dtype=np.float32)[None, :]
    tril = (jj <= ii).astype(NPBF16)
    masks = [
        np.concatenate([tril, np.ones((P, P), NPBF16)], axis=1),
        np.concatenate([tril, np.zeros((P, P), NPBF16)], axis=1),
    ]

    in_maps = []
    for c in range(N_CORES):
        b, r = divmod(c, 2)
        xp = x[b][_perm_rows(r), :]        # [S, D] fp32, permuted rows
        xpT = xp.T                          # [D, S]
        x8 = np.ascontiguousarray(xpT).astype(NPFP8)
        xT = np.ascontiguousarray(xpT[:, 0:JH]).astype(NPBF16)
        in_maps.append({
            "x8": x8, "xT": xT,
            "wq8": wq8, "wkT": wkT, "wvT": wvT,
            "mask": masks[r], "ones": ones,
        })
    return in_maps


def assemble_output(results):
    """Combine per-core partial (ctx, den) into the full [B, S, D] output."""
    out = np.empty((B, S, D), np.float32)
    for b in range(B):
        num = np.zeros((S, D), np.float32)
        den = np.zeros((S, 1), np.float32)
        for r in range(2):
            res = results[2 * b + r]
            ctx_p = np.asarray(res["ctx"]).astype(np.float32)   # [S, D] perm
            den_p = np.asarray(res["den"])                       # [P, 32]
            inv = _perm_rows(r)                # permuted pos -> global row
            num[inv] += ctx_p
            # den slot for permuted tile p: p = it*8 + ib, col = 2*(2*ib+it)
            dv = np.empty((S,), np.float32)
            for ib in range(N_IB):
                for it in range(2):
                    p_tile = it * 8 + ib
                    col = 2 * (2 * ib + it)
                    dv[p_tile * P:(p_tile + 1) * P] = den_p[:, col]
            den[inv, 0] += dv
        out[b] = num / den
    return out


def kernel(x, Wq, Wk, Wv):
    nc = _get_nc()
    in_maps = make_core_inputs(x, Wq, Wk, Wv)
    res = run_bass_kernel_spmd(nc, in_maps, list(range(N_CORES)))
    return assemble_output(res.results)
